# revision 1
# baseline (speedup 1.0000x reference)
"""Trainium2 Bass kernel for nn_DA_conv: per-sample dynamic depthwise 3x3 conv
(+LeakyReLU) followed by a 1x1 pointwise conv, with the 3x3 kernels produced by
a small per-sample MLP.

Strategy (8 NeuronCores, pure batch data-parallel, 2 samples per core):
  - SBUF layout: partition p = (sample s = p//64, channel c = p%64); the whole
    2-sample feature map lives resident in SBUF with zero-padded borders so
    every conv tap is a plain strided access-pattern read.
  - The kernel-generating MLP runs on the TensorEngine (tiny matmuls).
  - Depthwise 3x3 conv = 9 PSUM-accumulating diagonal matmuls per output tile.
    Diagonal 32x32 weight blocks + 32x32 TensorE array tiling (16 independent
    sub-tiles addressed via tile_position) recover the concurrency a depthwise
    contraction otherwise wastes on the 128x128 array.
  - LeakyReLU is fused into the PSUM->SBUF evacuation on the Scalar engine.
  - 1x1 conv = dense 32x32-tiled matmuls (contraction over channels), bias add
    fused into the PSUM->SBUF evacuation on the Vector engine.
  - Matmuls run in float32r (full-rate fp32 path; fp32 proper is 4x slower).
  - Emission is software-pipelined over half-blocks (depthwise of half m, then
    1x1 of half m-1) so PSUM evacuations overlap the next depthwise group.
"""

import os
import sys

sys.path.insert(0, "/opt/trn_rl_repo")

from contextlib import ExitStack

import numpy as np

import concourse.bacc as bacc
import concourse.bass as bass
import concourse.mybir as mybir
import concourse.tile as tile

S = 2            # samples per core
C = 64           # channels
H = W = 128      # spatial
KK = 3           # conv kernel size
NCORES = 8
RS = 132         # padded row stride in elements (16B-aligned: 132*4 = 528)
RP = H + 2       # padded row count (top/bottom halo)
XFREE = RP * RS  # padded image elements per partition
BR = 8           # image rows per block
NBLK = H // BR   # 16 blocks
HPX = (BR // 2) * W  # 512 pixels per half-block = one PSUM bank

f32 = mybir.dt.float32
f32r = mybir.dt.float32r
bf16 = mybir.dt.bfloat16
i32 = mybir.dt.int32

# x dtype for the depthwise matmuls. "f32r" keeps full fp32 DMA traffic;
# "bf16" halves the input DMA at a small accuracy cost.
X_MODE = os.environ.get("DA_CONV_X_MODE", "bf16")

LRELU = mybir.ActivationFunctionType.Lrelu
LRELU_MODE = os.environ.get("DA_CONV_LRELU", "prelu")
TAPS = [(di, dj) for di in range(KK) for dj in range(KK)]  # t = di*3 + dj


def build_program(x_mode: str = X_MODE) -> bass.Bass:
    # NOTE: fp32r matmuls cannot use TensorE column tiling on this toolchain
    # (s3d3_mm_valid_dst_partition), so the tiled conv stages must be bf16.
    xdt = bf16

    nc = bacc.Bacc("TRN2", target_bir_lowering=False, debug=False)

    x_d = nc.dram_tensor("x", [S * C, H * W], xdt, kind="ExternalInput").ap()
    dt_d = nc.dram_tensor("dT", [C, S], f32, kind="ExternalInput").ap()
    wk1_d = nc.dram_tensor("wk1t", [C, C], f32, kind="ExternalInput").ap()
    # Wk2 transposed + tap-major + duplicated over samples:
    # wk2td[j, t*128 + s*64 + c] = Wk2[c*9 + t, j]
    wk2_d = nc.dram_tensor("wk2td", [C, KK * KK * 2 * C], f32, kind="ExternalInput").ap()
    wct2_d = nc.dram_tensor("wct2", [2 * C, C], bf16, kind="ExternalInput").ap()
    bc_d = nc.dram_tensor("bc2", [2 * C, 1], f32, kind="ExternalInput").ap()
    out_d = nc.dram_tensor("out", [S * C, H * W], f32, kind="ExternalOutput").ap()

    with tile.TileContext(nc) as tc, ExitStack() as ctx:
        _body(ctx, tc, x_d, dt_d, wk1_d, wk2_d, wct2_d, bc_d, out_d, xdt)
    nc.compile()
    return nc


def _body(ctx, tc, x_d, dt_d, wk1_d, wk2_d, wct2_d, bc_d, out_d, xdt):
    nc = tc.nc
    const = ctx.enter_context(tc.tile_pool(name="const", bufs=1))
    xpool = ctx.enter_context(tc.tile_pool(name="xs", bufs=1))
    dwlp = ctx.enter_context(tc.tile_pool(name="dwl", bufs=4))
    abtp = ctx.enter_context(tc.tile_pool(name="abt", bufs=4))
    o2p = ctx.enter_context(tc.tile_pool(name="o2", bufs=NBLK // 2))
    pdw = ctx.enter_context(tc.tile_pool(name="pdw", bufs=2, space="PSUM"))
    po2 = ctx.enter_context(tc.tile_pool(name="po2", bufs=2, space="PSUM"))

    # ---------------- small-weight loads ----------------
    wk1t = const.tile([C, C], f32)
    nc.sync.dma_start(wk1t[:, :], wk1_d)
    wk2td = const.tile([C, KK * KK * 2 * C], f32)
    nc.sync.dma_start(wk2td[:, :], wk2_d)
    dts = const.tile([C, S], f32)
    nc.sync.dma_start(dts[:, :], dt_d)
    wct2 = const.tile([2 * C, C], bf16)
    nc.sync.dma_start(wct2[:, :], wct2_d)
    bc2 = const.tile([2 * C, 1], f32)
    nc.sync.dma_start(bc2[:, :], bc_d)

    # ---------------- kernel-generating MLP ----------------
    # hid[j, s] = lrelu(sum_i Wk1[j, i] d[s, i])  via lhsT = Wk1.T
    hid_ps = po2.tile([C, S], f32, tag="oe")
    nc.tensor.matmul(
        hid_ps[:, :], lhsT=wk1t[:, :], rhs=dts[:, :], start=True, stop=True,
    )
    hid_sb = const.tile([C, S], f32)
    if LRELU_MODE == "prelu":
        nc.scalar.activation(hid_sb[:, :], hid_ps[:, :],
                             mybir.ActivationFunctionType.Prelu, alpha=0.1)
    else:
        hid_ab = const.tile([C, S], f32)
        nc.scalar.activation(hid_ab[:, :], hid_ps[:, :],
                             mybir.ActivationFunctionType.Abs, scale=0.45)
        nc.vector.scalar_tensor_tensor(
            hid_sb[:, :], hid_ps[:, :], 0.55, hid_ab[:, :],
            op0=mybir.AluOpType.mult, op1=mybir.AluOpType.add,
        )

    # kern tap columns: kcols[s*64+c, t] = kern[s, c*9+t]
    kcols = const.tile([2 * C, KK * KK], f32)
    for t in range(KK * KK):
        kp = po2.tile([2 * C, S], f32, tag="oe")
        nc.tensor.matmul(
            kp[:, :],
            lhsT=wk2td[:, t * 128 : (t + 1) * 128],
            rhs=hid_sb[:, :],
            start=True, stop=True,
        )
        # partition p wants free column s = p//64 of kp (partition-aligned copies)
        nc.vector.tensor_copy(kcols[0:C, t : t + 1], kp[0:C, 0:1])
        nc.vector.tensor_copy(kcols[C : 2 * C, t : t + 1], kp[C : 2 * C, 1:2])

    # identity -> per-tap diagonal weight matrices diag[:, t*128:(t+1)*128]
    id_i = const.tile([128, 128], i32)
    nc.gpsimd.iota(id_i[:, :], pattern=[[1, 128]], base=0, channel_multiplier=-1)
    idf = const.tile([128, 128], f32)
    nc.vector.tensor_scalar(idf[:, :], id_i[:, :], 0, None, mybir.AluOpType.is_equal)
    diag = const.tile([128, KK * KK * 128], xdt)
    for t in range(KK * KK):
        nc.vector.tensor_scalar_mul(
            diag[:, t * 128 : (t + 1) * 128], idf[:, :], kcols[:, t : t + 1]
        )

    # ---------------- resident padded feature map ----------------
    xs = xpool.tile([128, XFREE], xdt)
    # top halo row + row-1 left pad (contiguous), bottom halo row, and the
    # pad columns: right-pad of row r is contiguous with left-pad of row r+1,
    # so one strided memset covers all interior pad columns.
    nc.vector.memset(xs[:, 0 : RS + 1], 0.0)
    nc.vector.memset(xs[:, (RP - 1) * RS : RP * RS], 0.0)
    pads = xs[:, W + 1 : W + 1 + (H + 1) * RS].rearrange("p (r w) -> p r w", w=RS)
    nc.vector.memset(pads[:, :, 0:4], 0.0)
    # image rows in 16 chunks so compute can start early
    for k in range(NBLK):
        src = x_d[:, k * BR * W : (k + 1) * BR * W].rearrange(
            "p (r w) -> p r w", w=W
        )
        o = (k * BR + 1) * RS + 1
        dst = xs[:, o : o + BR * RS].rearrange("p (r w) -> p r w", w=RS)[:, :, 0:W]
        nc.sync.dma_start(dst, src)

    # ---------------- main loop ----------------
    # 64x64 TensorE tiling: 4 concurrent positions. Each PSUM bank has exactly
    # one row-tile writer (HW constraint): P_A <- row tile 0 (sample A
    # channels), P_B <- row tile 1; column groups select the pixel half (E =
    # rows 8k..8k+3, O = rows 8k+4..8k+7) within the bank.
    xrows = xs[:, :].rearrange("p (r w) -> p r w", w=RS)

    def lrelu_evac(D, P):
        if LRELU_MODE == "prelu":
            nc.scalar.activation(D[:, :], P[:, :],
                                 mybir.ActivationFunctionType.Prelu, alpha=0.1)
        else:
            # lrelu(x) = 0.55x + 0.45|x| ; Abs on ScalarE, fused MAC on VectorE
            ab = abtp.tile([128, HPX], f32, tag="abt")
            nc.scalar.activation(ab[:, :], P[:, :],
                                 mybir.ActivationFunctionType.Abs, scale=0.45)
            nc.vector.scalar_tensor_tensor(
                D[:, :], P[:, :], 0.55, ab[:, :],
                op0=mybir.AluOpType.mult, op1=mybir.AluOpType.add,
            )

    def dw_stage(k):
        r0e = BR * k
        r0o = BR * k + BR // 2
        PA = pdw.tile([128, HPX], f32, tag="pa")
        PB = pdw.tile([128, HPX], f32, tag="pb")
        for t, (di, dj) in enumerate(TAPS):
            wE = xrows[:, r0e + di : r0e + di + 4, dj : dj + W]
            wO = xrows[:, r0o + di : r0o + di + 4, dj : dj + W]
            la = diag[0:C, t * 128 : t * 128 + C]
            lb = diag[C : 2 * C, t * 128 + C : t * 128 + 2 * C]
            for cg, win in ((0, wE), (C, wO)):
                nc.tensor.matmul(
                    PA[cg : cg + C, :], lhsT=la, rhs=win[0:C, :, :],
                    start=(t == 0), stop=(t == KK * KK - 1),
                    tile_position=(0, cg), skip_group_check=True,
                )
                nc.tensor.matmul(
                    PB[cg : cg + C, :], lhsT=lb, rhs=win[C : 2 * C, :, :],
                    start=(t == 0), stop=(t == KK * KK - 1),
                    tile_position=(C, cg), skip_group_check=True,
                )
        DA = dwlp.tile([128, HPX], bf16, tag="da")
        DB = dwlp.tile([128, HPX], bf16, tag="db")
        lrelu_evac(DA, PA)
        lrelu_evac(DB, PB)
        return k, DA, DB

    def conv1x1_stage(k, DA, DB):
        OE = po2.tile([128, HPX], f32, tag="oe")
        OO = po2.tile([128, HPX], f32, tag="oo")
        # E outputs via row tile 0, O outputs via row tile 1; standard [A;B]
        # channel layout lands directly in each output bank.
        nc.tensor.matmul(OE[0:C, :], lhsT=wct2[0:C, :], rhs=DA[0:C, :],
                         start=True, stop=True, tile_position=(0, 0),
                         skip_group_check=True)
        nc.tensor.matmul(OE[C : 2 * C, :], lhsT=wct2[0:C, :], rhs=DB[0:C, :],
                         start=True, stop=True, tile_position=(0, C),
                         skip_group_check=True)
        nc.tensor.matmul(OO[0:C, :], lhsT=wct2[C : 2 * C, :], rhs=DA[C : 2 * C, :],
                         start=True, stop=True, tile_position=(C, 0),
                         skip_group_check=True)
        nc.tensor.matmul(OO[C : 2 * C, :], lhsT=wct2[C : 2 * C, :],
                         rhs=DB[C : 2 * C, :],
                         start=True, stop=True, tile_position=(C, C),
                         skip_group_check=True)
        # bias add into the (128, 2048) staging tile; 1 MiB output DMA / 2 blocks
        q, qi = divmod(k, 2)
        if qi == 0:
            zcur["t"] = o2p.tile([128, 4 * HPX], f32, tag="o2", name=f"zt{k}")
        zt = zcur["t"]
        zb = 2 * qi * HPX
        nc.vector.tensor_scalar_add(zt[:, zb : zb + HPX], OE[:, :], bc2[:, 0:1])
        nc.vector.tensor_scalar_add(
            zt[:, zb + HPX : zb + 2 * HPX], OO[:, :], bc2[:, 0:1]
        )
        if qi == 1:
            nc.sync.dma_start(out_d[:, q * 4 * HPX : (q + 1) * 4 * HPX], zt[:, :])

    pending = None
    zcur = {"t": None}
    for k in range(NBLK):
        st = dw_stage(k)
        if pending is not None:
            conv1x1_stage(*pending)
        pending = st
    conv1x1_stage(*pending)


# ---------------------------------------------------------------------------
# host-side entry point
# ---------------------------------------------------------------------------

_PROGRAM_CACHE: dict[str, bass.Bass] = {}


def _get_program(x_mode: str) -> bass.Bass:
    if x_mode not in _PROGRAM_CACHE:
        _PROGRAM_CACHE[x_mode] = build_program(x_mode)
    return _PROGRAM_CACHE[x_mode]


def _host_prep(inputs: dict, x_mode: str):
    import ml_dtypes

    x = np.asarray(inputs["x"], dtype=np.float32)
    d = np.asarray(inputs["d"], dtype=np.float32)
    Wk1 = np.asarray(inputs["Wk1"], dtype=np.float32)
    Wk2 = np.asarray(inputs["Wk2"], dtype=np.float32)
    Wc = np.asarray(inputs["Wc"], dtype=np.float32)
    bc = np.asarray(inputs["bc"], dtype=np.float32)

    wk1t = np.ascontiguousarray(Wk1.T)
    w = Wk2.reshape(C, KK * KK, C).transpose(2, 1, 0)  # (j, t, c)
    wk2td = np.ascontiguousarray(
        np.concatenate([w, w], axis=2).reshape(C, KK * KK * 2 * C)
    )
    wct = np.ascontiguousarray(Wc.T)
    wct2 = np.ascontiguousarray(np.concatenate([wct, wct], axis=0)).astype(
        ml_dtypes.bfloat16
    )
    bc2 = np.ascontiguousarray(np.concatenate([bc, bc]).reshape(2 * C, 1))

    xcast = x.astype(ml_dtypes.bfloat16)

    in_maps = []
    for i in range(NCORES):
        xs = np.ascontiguousarray(xcast[S * i : S * (i + 1)].reshape(S * C, H * W))
        dT = np.ascontiguousarray(d[S * i : S * (i + 1)].T)
        in_maps.append(
            {
                "x": xs,
                "dT": dT,
                "wk1t": wk1t,
                "wk2td": wk2td,
                "wct2": wct2,
                "bc2": bc2,
            }
        )
    return in_maps


def run_on_hw(inputs: dict, x_mode: str = None, **kwargs):
    """Run the SPMD kernel on 8 NeuronCores; returns (output, BassKernelResults)."""
    from concourse.bass_utils import run_bass_kernel_spmd

    x_mode = x_mode or X_MODE
    nc = _get_program(x_mode)
    in_maps = _host_prep(inputs, x_mode)
    res = run_bass_kernel_spmd(nc, in_maps, core_ids=list(range(NCORES)), **kwargs)
    outs = res.results
    B = S * NCORES
    out = np.empty((B, C, H, W), dtype=np.float32)
    for i in range(NCORES):
        out[S * i : S * (i + 1)] = outs[i]["out"].reshape(S, C, H, W)
    return out, res


def kernel(**inputs) -> np.ndarray:
    out, _ = run_on_hw(inputs)
    return out


if __name__ == "__main__":
    nc = build_program()
    print("program built OK")



# revision 4
# speedup vs baseline: 2.7474x; 2.7474x over previous
"""Trainium2 Bass kernel for nn_DA_conv: per-sample dynamic depthwise 3x3 conv
(+LeakyReLU) followed by a 1x1 pointwise conv, with the 3x3 kernels produced by
a small per-sample MLP.

Strategy (8 NeuronCores, batch data-parallel, 2 samples per core):
  - SBUF partition p = (sample s = p//64, channel c = p%64); the feature map
    is resident in bf16 with top/bottom halo rows plus a one-element pad on
    each end (no W padding: edge columns are handled by full-width taps plus
    strided correction matmuls / partial-width vector ops).
  - The image rows are split into three regions, one per compute engine, so
    the depthwise conv runs on PE, DVE and GPSIMD concurrently:
      A  : PE diagonal matmuls (9 full-width PSUM-accumulated taps + 6
           strided W-edge corrections per 4-row bank), ACT Prelu evac.
      B2 : DVE per-tap scale-muls (4x mode) + tensor-tensor adds.
      B3 : gpsimd scalar_tensor_tensor MAC chain.
    LeakyReLU for B2/B3 runs on ACT (Prelu), interleaved by estimated
    readiness so the in-order ACT queue never stalls the PSUM pipeline.
  - 1x1 conv: one 128x128 block-diagonal matmul per PSUM bank covers both
    samples at once; ACT evacuates with the bias via Identity+bias.
  - DMA: input + output both on the otherwise-idle SP queue; compute engines
    never issue DMAs.
"""

import sys

sys.path.insert(0, "/opt/trn_rl_repo")

from contextlib import ExitStack

import numpy as np

import concourse.bacc as bacc
import concourse.bass as bass
import concourse.mybir as mybir
import concourse.tile as tile

S = 2            # samples per core
C = 64           # channels
H = W = 128      # spatial
KK = 3           # conv kernel size
NCORES = 8
XR = H + 2       # resident rows incl halo

f32 = mybir.dt.float32
bf16 = mybir.dt.bfloat16
A = mybir.AluOpType
PRELU = mybir.ActivationFunctionType.Prelu
IDENT = mybir.ActivationFunctionType.Identity

# Row stripes per engine (cover 0..128, multiples of 4).
# Interleaved so 1x1-group completions stagger instead of bunching at the end.
STRIPES_A = [(28, 16), (0, 16), (56, 16), (84, 16), (124, 4)]   # PE, 68 rows
STRIPES_B2 = [(16, 12), (44, 12), (112, 12), (72, 4)]           # DVE, 40 rows
STRIPES_B3 = [(100, 12), (76, 8)]                               # Pool, 20 rows
R_A = sum(n for _, n in STRIPES_A)
R_B2 = sum(n for _, n in STRIPES_B2)
R_B3 = sum(n for _, n in STRIPES_B3)

# virtual-clock pace estimates (ns per image row) for emission ordering
PACE_A = 490.0
PACE_B2 = 890.0
PACE_B3 = 1850.0

TAPS = [(di, dj) for di in range(KK) for dj in range(KK)]
CORR = [(di, dj) for dj in (0, 2) for di in range(KK)]


def _tap_geom(dj):
    """(out_col_lo, src_col_lo, ncols) for a horizontal tap shift."""
    if dj == 0:
        return 1, 0, W - 1
    if dj == 1:
        return 0, 0, W
    return 0, 1, W - 1


def build_program() -> bass.Bass:
    nc = bacc.Bacc("TRN2", target_bir_lowering=False, debug=False)

    x_d = nc.dram_tensor("x", [S * C, H * W], bf16, kind="ExternalInput").ap()
    # mlpw = [dT (2) | Wk1.T (64) | wk2td (1152)] in one tensor -> one DMA
    # (wk2td[j, t*128 + s*64 + c] = Wk2[c*9 + t, j], tap-major, sample-dup)
    mlpw_d = nc.dram_tensor(
        "mlpw", [C, S + C + KK * KK * 2 * C], bf16, kind="ExternalInput"
    ).ap()
    wcbd_d = nc.dram_tensor("wcbd", [2 * C, 2 * C], bf16, kind="ExternalInput").ap()
    bc_d = nc.dram_tensor("bc2", [2 * C, 1], f32, kind="ExternalInput").ap()
    out_d = nc.dram_tensor("out", [S * C, H * W], bf16, kind="ExternalOutput").ap()

    with tile.TileContext(nc) as tc, ExitStack() as ctx:
        _body(ctx, tc, x_d, mlpw_d, wcbd_d, bc_d, out_d)
    nc.compile()
    return nc


def _body(ctx, tc, x_d, mlpw_d, wcbd_d, bc_d, out_d):
    nc = tc.nc
    const = ctx.enter_context(tc.tile_pool(name="const", bufs=1))
    xpool = ctx.enter_context(tc.tile_pool(name="xs", bufs=1))
    dwp = ctx.enter_context(tc.tile_pool(name="dw", bufs=1))
    tmpp = ctx.enter_context(tc.tile_pool(name="tmp", bufs=2))
    sbp = ctx.enter_context(tc.tile_pool(name="sb", bufs=2))
    obp = ctx.enter_context(tc.tile_pool(name="ob", bufs=3))
    pdw = ctx.enter_context(tc.tile_pool(name="pdw", bufs=2, space="PSUM"))
    po = ctx.enter_context(tc.tile_pool(name="po", bufs=3, space="PSUM"))

    # ---------------- const loads (qSP, before x) ----------------
    mlpw = const.tile([C, S + C + KK * KK * 2 * C], bf16)
    nc.sync.dma_start(mlpw[:, :], mlpw_d)
    dts = mlpw[:, 0:S]
    wk1t = mlpw[:, S : S + C]
    wk2td = mlpw[:, S + C : S + C + KK * KK * 2 * C]

    # ---------------- resident x (halo rows zeroed, +-1 elem pad) ----------
    xs = xpool.tile([128, XR * W + 2], bf16)
    nc.vector.memset(xs[:, 0 : W + 1], 0.0)
    nc.vector.memset(xs[:, (XR - 1) * W + 1 : XR * W + 2], 0.0)
    XBLK = 8

    def _xchunks():
        """order the 16 aligned 8-row x chunks by estimated consume time."""
        consume_at = {}
        for stripes, pace in ((STRIPES_A, PACE_A), (STRIPES_B2, PACE_B2),
                              (STRIPES_B3, PACE_B3)):
            t = 0.0
            for r0, n in stripes:
                for r in range(r0, r0 + n):
                    consume_at[r] = t + pace * (r - r0)
                t += pace * n

        def consume(k):
            rows = range(max(0, k - 1), min(H, k + XBLK + 1))
            return min(consume_at[r] for r in rows)

        return sorted(range(0, H, XBLK), key=consume)

    wcbd = const.tile([2 * C, 2 * C], bf16)
    bc2 = const.tile([2 * C, 1], f32)
    _xorder = _xchunks()
    # SP carries the early-consumed chunks now; ACT takes the rest after the
    # MLP section (so the in-order ACT queue does the MLP prelu first).
    _xsp = [k for ci, k in enumerate(_xorder) if ci < 6 or ci % 5 != 1]
    _xact = [k for k in _xorder if k not in _xsp]
    for ci, k in enumerate(_xsp):
        nc.sync.dma_start(
            xs[:, (k + 1) * W + 1 : (k + XBLK + 1) * W + 1],
            x_d[:, k * W : (k + XBLK) * W],
        )
        if ci == 3:
            nc.sync.dma_start(wcbd[:, :], wcbd_d)
            nc.sync.dma_start(bc2[:, :], bc_d)

    # dw output (lrelu'd, bf16) == rhs of the 1x1 conv
    dwout = dwp.tile([128, H * W], bf16)



    # ---------------- kernel-generating MLP ----------------
    hid_ps = po.tile([C, S], f32, tag="po")
    nc.tensor.matmul(hid_ps[:, :], lhsT=wk1t, rhs=dts, start=True, stop=True)
    hid_sb = const.tile([C, S], bf16)
    nc.scalar.activation(hid_sb[:, :], hid_ps[:, :], PRELU, alpha=0.1)

    kcols = const.tile([2 * C, KK * KK], f32)
    kps = po.tile([2 * C, KK * KK], f32, tag="po")
    for t in range(KK * KK):
        nc.tensor.matmul(
            kps[0:C, t : t + 1],
            lhsT=wk2td[:, t * 128 : t * 128 + C],
            rhs=hid_sb[:, 0:1],
            start=True, stop=True,
        )
        nc.tensor.matmul(
            kps[C : 2 * C, t : t + 1],
            lhsT=wk2td[:, t * 128 + C : (t + 1) * 128],
            rhs=hid_sb[:, 1:2],
            start=True, stop=True,
        )
    nc.vector.tensor_copy(kcols[:, :], kps[:, :])

    # per-tap diagonal weights for the PE region
    id_i = const.tile([128, 128], mybir.dt.int32)
    nc.gpsimd.iota(id_i[:, :], pattern=[[1, 128]], base=0, channel_multiplier=-1)
    idf = const.tile([128, 128], bf16)
    nc.vector.tensor_scalar(idf[:, :], id_i[:, :], 0, None, A.is_equal)
    diag = const.tile([128, KK * KK * 128], bf16)
    for t in range(KK * KK):
        nc.vector.tensor_scalar_mul(
            diag[:, t * 128 : (t + 1) * 128], idf[:, :], kcols[:, t : t + 1]
        )
    # negated diagonals for the W-edge wrap corrections (taps dj=0 and dj=2)
    negk = const.tile([2 * C, KK * KK], f32)
    nc.vector.tensor_scalar_mul(negk[:, :], kcols[:, :], -1.0)
    negdiag = const.tile([128, 6 * 128], bf16)
    for j, (di, dj) in enumerate(CORR):
        t = di * KK + dj
        nc.vector.tensor_scalar_mul(
            negdiag[:, j * 128 : (j + 1) * 128], idf[:, :], negk[:, t : t + 1]
        )

    for k in _xact:
        nc.scalar.dma_start(
            xs[:, (k + 1) * W + 1 : (k + XBLK + 1) * W + 1],
            x_d[:, k * W : (k + XBLK) * W],
        )

    xs3 = xs[:, 1 : 1 + XR * W].rearrange("p (r w) -> p r w", w=W)  # [128, XR, W]
    dw3 = dwout[:, :].rearrange("p (r w) -> p r w", w=W)

    # ---------------- region emitters ----------------
    def emit_A():
        """PE: 9 full-width taps + 6 strided W-edge corrections per 4-row
        bank; ACT Prelu evacuates to dwout."""
        banks = [r0 + 4 * i for r0, n in STRIPES_A for i in range(n // 4)]
        for rb in banks:
            if True:
                ps = pdw.tile([128, 512], f32, tag="pdw", name=f"pdw{rb}")
                bank = ps[:, :]
                for i, (di, dj) in enumerate(TAPS[:-1]):
                    t = di * KK + dj
                    base = (rb + di) * W + dj
                    nc.tensor.matmul(
                        bank[:, :],
                        lhsT=diag[:, t * 128 : (t + 1) * 128],
                        rhs=xs[:, base : base + 512],
                        start=(i == 0),
                        stop=False,
                    )
                for j, (di, dj) in enumerate(CORR):
                    if dj == 0:
                        dst = bank[:, 0 : 512 : W]
                        c0 = (rb + di) * W
                    else:
                        dst = bank[:, W - 1 : 512 : W]
                        c0 = (rb + di + 1) * W + 1
                    nc.tensor.matmul(
                        dst,
                        lhsT=negdiag[:, j * 128 : (j + 1) * 128],
                        rhs=xs[:, c0 : c0 + 3 * W + 1 : W],
                        start=False,
                        stop=False,
                    )
                di, dj = TAPS[-1]
                t = di * KK + dj
                base = (rb + di) * W + dj
                nc.tensor.matmul(
                    bank[:, :],
                    lhsT=diag[:, t * 128 : (t + 1) * 128],
                    rhs=xs[:, base : base + 512],
                    start=False,
                    stop=True,
                )
            nc.scalar.activation(
                dwout[:, rb * W : (rb + 4) * W], ps[:, :], PRELU, alpha=0.1
            )
            yield rb, 4

    def emit_B2():
        """DVE: scale-mul into tmp (4x), tensor-tensor add into dwout (2x);
        lrelu on ACT."""
        for ci, (r0, n) in enumerate(STRIPES_B2):
            last = ci == len(STRIPES_B2) - 1
            dst = dwout[:, r0 * W : (r0 + n) * W]
            t0 = 0 * KK + 1
            nc.vector.tensor_scalar_mul(
                dst, xs[:, r0 * W + 1 : (r0 + n) * W + 1], kcols[:, t0 : t0 + 1]
            )
            for di, dj in TAPS:
                if (di, dj) == (0, 1):
                    continue
                t = di * KK + dj
                w_lo, s_lo, ncol = _tap_geom(dj)
                tm = tmpp.tile([128, 16 * W], bf16, tag="b2t", name=f"tm{r0}_{t}")
                tm3 = tm[:, :].rearrange("p (r w) -> p r w", w=W)
                src = xs3[:, r0 + di : r0 + di + n, s_lo : s_lo + ncol]
                nc.vector.tensor_scalar_mul(tm3[:, 0:n, 0:ncol], src, kcols[:, t : t + 1])
                nc.vector.tensor_tensor(
                    dw3[:, r0 : r0 + n, w_lo : w_lo + ncol],
                    dw3[:, r0 : r0 + n, w_lo : w_lo + ncol],
                    tm3[:, 0:n, 0:ncol],
                    A.add,
                )
            nc.scalar.activation(dst, dst, PRELU, alpha=0.1)
            yield r0, n

    def emit_B3():
        """Pool: scale-mul into tmp (TensorScalarPtr) + tensor-tensor add
        into dwout; lrelu on ACT."""
        B3MAX = max(n for _, n in STRIPES_B3)
        for ci, (r0, n) in enumerate(STRIPES_B3):
            dst = dwout[:, r0 * W : (r0 + n) * W]
            t0 = 0 * KK + 1
            nc.gpsimd.tensor_scalar_mul(
                dst, xs[:, r0 * W + 1 : (r0 + n) * W + 1], kcols[:, t0 : t0 + 1]
            )
            for di, dj in TAPS:
                if (di, dj) == (0, 1):
                    continue
                t = di * KK + dj
                w_lo, s_lo, ncol = _tap_geom(dj)
                pt = sbp.tile([128, B3MAX * W], bf16, tag="p3t", name=f"pt{r0}_{t}")
                pt3 = pt[:, :].rearrange("p (r w) -> p r w", w=W)
                src = xs3[:, r0 + di : r0 + di + n, s_lo : s_lo + ncol]
                nc.gpsimd.tensor_scalar_mul(
                    pt3[:, 0:n, 0:ncol], src, kcols[:, t : t + 1]
                )
                nc.gpsimd.tensor_tensor(
                    dw3[:, r0 : r0 + n, w_lo : w_lo + ncol],
                    dw3[:, r0 : r0 + n, w_lo : w_lo + ncol],
                    pt3[:, 0:n, 0:ncol],
                    A.add,
                )
            nc.scalar.activation(dst, dst, PRELU, alpha=0.1)
            yield r0, n

    # ------- merged emission: dw chunks + 1x1 groups by virtual clocks -----
    def emit_group(g):
        rb = g * 8
        ps1 = po.tile([128, 1024], f32, tag="po", name=f"po{g}")
        for half in range(2):
            off = (rb + 4 * half) * W
            nc.tensor.matmul(
                ps1[:, half * 512 : (half + 1) * 512],
                lhsT=wcbd[:, :],
                rhs=dwout[:, off : off + 512],
                start=True, stop=True,
            )
        ob = obp.tile([128, 1024], bf16, tag="ob", name=f"ob{g}")
        emit_group.count += 1
        nc.scalar.activation(ob[:, :], ps1[:, :], IDENT, bias=bc2[:, 0:1])
        nc.sync.dma_start(out_d[:, rb * W : (rb + 8) * W], ob[:, :])

    emit_group.count = 0
    FILL = 3000.0  # rough ns before first dw work can start
    regions = [
        (emit_A(), PACE_A),
        (emit_B2(), PACE_B2),
        (emit_B3(), PACE_B3),
    ]
    clocks = [FILL] * len(regions)
    done_at: dict[int, float] = {}  # 4-row unit index -> est completion
    pending: list[tuple[float, int]] = []  # (ready est, 1x1 group)
    queued: set[int] = set()
    idx = list(range(len(regions)))
    while idx or pending:
        tnext = min((clocks[j] for j in idx), default=1e18)
        if pending and pending[0][0] <= tnext:
            _, g = pending.pop(0)
            emit_group(g)
            continue
        if not idx:
            continue
        i = min(idx, key=lambda j: clocks[j])
        ggen, pace = regions[i]
        try:
            r0, n = next(ggen)
        except StopIteration:
            idx.remove(i)
            continue
        assert r0 % 4 == 0 and n % 4 == 0
        clocks[i] += pace * n
        for u in range(r0 // 4, (r0 + n) // 4):
            done_at[u] = clocks[i]
        for g in range(H // 8):
            if g in queued:
                continue
            if 2 * g in done_at and 2 * g + 1 in done_at:
                queued.add(g)
                pending.append((max(done_at[2 * g], done_at[2 * g + 1]), g))
        pending.sort()
    assert len(queued) == H // 8, f"unemitted 1x1 groups: {sorted(set(range(16)) - queued)}"


# ---------------------------------------------------------------------------
# host-side entry point
# ---------------------------------------------------------------------------

_PROGRAM_CACHE: dict[str, bass.Bass] = {}


def _get_program() -> bass.Bass:
    if "p" not in _PROGRAM_CACHE:
        _PROGRAM_CACHE["p"] = build_program()
    return _PROGRAM_CACHE["p"]


def _host_prep(inputs: dict):
    import ml_dtypes

    x = np.asarray(inputs["x"], dtype=np.float32)
    d = np.asarray(inputs["d"], dtype=np.float32)
    Wk1 = np.asarray(inputs["Wk1"], dtype=np.float32)
    Wk2 = np.asarray(inputs["Wk2"], dtype=np.float32)
    Wc = np.asarray(inputs["Wc"], dtype=np.float32)
    bc = np.asarray(inputs["bc"], dtype=np.float32)

    wk1t = np.ascontiguousarray(Wk1.T)
    w = Wk2.reshape(C, KK * KK, C).transpose(2, 1, 0)  # (j, t, c)
    wk2td = np.ascontiguousarray(
        np.concatenate([w, w], axis=2).reshape(C, KK * KK * 2 * C)
    )
    wct = np.ascontiguousarray(Wc.T)
    wcbd = np.zeros((2 * C, 2 * C), np.float32)
    wcbd[0:C, 0:C] = wct
    wcbd[C:, C:] = wct
    wcbd = wcbd.astype(ml_dtypes.bfloat16)
    bc2 = np.ascontiguousarray(np.concatenate([bc, bc]).reshape(2 * C, 1))

    xcast = x.astype(ml_dtypes.bfloat16)

    in_maps = []
    for i in range(NCORES):
        xsamp = np.ascontiguousarray(xcast[S * i : S * (i + 1)].reshape(S * C, H * W))
        dT = np.ascontiguousarray(d[S * i : S * (i + 1)].T)
        mlpw = np.concatenate([dT, wk1t, wk2td], axis=1).astype(ml_dtypes.bfloat16)
        in_maps.append(
            {
                "x": xsamp,
                "mlpw": np.ascontiguousarray(mlpw),
                "wcbd": wcbd,
                "bc2": bc2,
            }
        )
    return in_maps


def run_on_hw(inputs: dict, **kwargs):
    from concourse.bass_utils import run_bass_kernel_spmd

    nc = _get_program()
    in_maps = _host_prep(inputs)
    res = run_bass_kernel_spmd(nc, in_maps, core_ids=list(range(NCORES)), **kwargs)
    outs = res.results
    B = S * NCORES
    out = np.empty((B, C, H, W), dtype=np.float32)
    for i in range(NCORES):
        out[S * i : S * (i + 1)] = (
            outs[i]["out"].astype(np.float32).reshape(S, C, H, W)
        )
    return out, res


def kernel(**inputs) -> np.ndarray:
    out, _ = run_on_hw(inputs)
    return out


if __name__ == "__main__":
    nc = build_program()
    print("program built OK")


# revision 5
# speedup vs baseline: 2.8063x; 1.0215x over previous
"""Trainium2 Bass kernel for nn_DA_conv: per-sample dynamic depthwise 3x3 conv
(+LeakyReLU) followed by a 1x1 pointwise conv, with the 3x3 kernels produced by
a small per-sample MLP.

Strategy (8 NeuronCores, batch data-parallel, 2 samples per core):
  - SBUF partition p = (sample s = p//64, channel c = p%64); the feature map
    is resident in bf16 with top/bottom halo rows plus a one-element pad on
    each end (no W padding: edge columns are handled by full-width taps plus
    strided correction matmuls / partial-width vector ops).
  - The image rows are split into three regions, one per compute engine, so
    the depthwise conv runs on PE, DVE and GPSIMD concurrently:
      A  : PE diagonal matmuls (9 full-width PSUM-accumulated taps + 6
           strided W-edge corrections per 4-row bank), ACT Prelu evac.
      B2 : DVE per-tap scale-muls (4x mode) + tensor-tensor adds.
      B3 : gpsimd scalar_tensor_tensor MAC chain.
    LeakyReLU for B2/B3 runs on ACT (Prelu), interleaved by estimated
    readiness so the in-order ACT queue never stalls the PSUM pipeline.
  - 1x1 conv: one 128x128 block-diagonal matmul per PSUM bank covers both
    samples at once; ACT evacuates with the bias via Identity+bias.
  - DMA: input + output both on the otherwise-idle SP queue; compute engines
    never issue DMAs.
"""

import sys

sys.path.insert(0, "/opt/trn_rl_repo")

from contextlib import ExitStack

import numpy as np

import concourse.bacc as bacc
import concourse.bass as bass
import concourse.mybir as mybir
import concourse.tile as tile

S = 2            # samples per core
C = 64           # channels
H = W = 128      # spatial
KK = 3           # conv kernel size
NCORES = 8
XR = H + 2       # resident rows incl halo

f32 = mybir.dt.float32
bf16 = mybir.dt.bfloat16
A = mybir.AluOpType
PRELU = mybir.ActivationFunctionType.Prelu
IDENT = mybir.ActivationFunctionType.Identity

# Row stripes per engine (cover 0..128, multiples of 4).
# Interleaved so 1x1-group completions stagger instead of bunching at the end.
STRIPES_A = [(0, 16), (28, 16), (56, 16), (84, 16), (124, 4)]   # PE, 68 rows
STRIPES_B2 = [(44, 12), (16, 12), (112, 12), (72, 4)]           # DVE, 40 rows
STRIPES_B3 = [(100, 12), (76, 8)]                               # Pool, 20 rows
R_A = sum(n for _, n in STRIPES_A)
R_B2 = sum(n for _, n in STRIPES_B2)
R_B3 = sum(n for _, n in STRIPES_B3)

# virtual-clock pace estimates (ns per image row) for emission ordering
PACE_A = 490.0
PACE_B2 = 890.0
PACE_B3 = 1850.0
K_FILL = 3000.0

TAPS = [(di, dj) for di in range(KK) for dj in range(KK)]
CORR = [(di, dj) for dj in (0, 2) for di in range(KK)]


def _tap_geom(dj):
    """(out_col_lo, src_col_lo, ncols) for a horizontal tap shift."""
    if dj == 0:
        return 1, 0, W - 1
    if dj == 1:
        return 0, 0, W
    return 0, 1, W - 1


def build_program() -> bass.Bass:
    nc = bacc.Bacc("TRN2", target_bir_lowering=False, debug=False)

    x_d = nc.dram_tensor("x", [S * C, H * W], bf16, kind="ExternalInput").ap()
    # mlpw = [dT (2) | Wk1.T (64) | wk2td (1152)] in one tensor -> one DMA
    # (wk2td[j, t*128 + s*64 + c] = Wk2[c*9 + t, j], tap-major, sample-dup)
    mlpw_d = nc.dram_tensor(
        "mlpw", [C, S + C + KK * KK * 2 * C], bf16, kind="ExternalInput"
    ).ap()
    wcbd_d = nc.dram_tensor("wcbd", [2 * C, 2 * C], bf16, kind="ExternalInput").ap()
    bc_d = nc.dram_tensor("bc2", [2 * C, 1], f32, kind="ExternalInput").ap()
    out_d = nc.dram_tensor("out", [S * C, H * W], bf16, kind="ExternalOutput").ap()

    with tile.TileContext(nc) as tc, ExitStack() as ctx:
        _body(ctx, tc, x_d, mlpw_d, wcbd_d, bc_d, out_d)
    nc.compile()
    return nc


def _body(ctx, tc, x_d, mlpw_d, wcbd_d, bc_d, out_d):
    nc = tc.nc
    const = ctx.enter_context(tc.tile_pool(name="const", bufs=1))
    xpool = ctx.enter_context(tc.tile_pool(name="xs", bufs=1))
    dwp = ctx.enter_context(tc.tile_pool(name="dw", bufs=1))
    tmpp = ctx.enter_context(tc.tile_pool(name="tmp", bufs=2))
    sbp = ctx.enter_context(tc.tile_pool(name="sb", bufs=2))
    obp = ctx.enter_context(tc.tile_pool(name="ob", bufs=3))
    pdw = ctx.enter_context(tc.tile_pool(name="pdw", bufs=2, space="PSUM"))
    po = ctx.enter_context(tc.tile_pool(name="po", bufs=3, space="PSUM"))

    # ---------------- const loads (qSP, before x) ----------------
    mlpw = const.tile([C, S + C + KK * KK * 2 * C], bf16)
    nc.sync.dma_start(mlpw[:, :], mlpw_d)
    dts = mlpw[:, 0:S]
    wk1t = mlpw[:, S : S + C]
    wk2td = mlpw[:, S + C : S + C + KK * KK * 2 * C]

    # ---------------- resident x (halo rows zeroed, +-1 elem pad) ----------
    xs = xpool.tile([128, XR * W + 2], bf16)
    nc.vector.memset(xs[:, 0 : W + 1], 0.0)
    nc.vector.memset(xs[:, (XR - 1) * W + 1 : XR * W + 2], 0.0)
    XBLK = 8

    def _xchunks():
        """order the 16 aligned 8-row x chunks by estimated consume time."""
        consume_at = {}
        for stripes, pace in ((STRIPES_A, PACE_A), (STRIPES_B2, PACE_B2),
                              (STRIPES_B3, PACE_B3)):
            t = 0.0
            for r0, n in stripes:
                for r in range(r0, r0 + n):
                    consume_at[r] = t + pace * (r - r0)
                t += pace * n

        def consume(k):
            rows = range(max(0, k - 1), min(H, k + XBLK + 1))
            return min(consume_at[r] for r in rows)

        return sorted(range(0, H, XBLK), key=consume)

    wcbd = const.tile([2 * C, 2 * C], bf16)
    bc2 = const.tile([2 * C, 1], f32)
    _xorder = _xchunks()
    # SP carries the early-consumed chunks now; ACT takes the rest after the
    # MLP section (so the in-order ACT queue does the MLP prelu first).
    _xsp = [k for ci, k in enumerate(_xorder) if ci < 6 or ci % 5 != 1]
    _xact = [k for k in _xorder if k not in _xsp]
    for ci, k in enumerate(_xsp):
        nc.sync.dma_start(
            xs[:, (k + 1) * W + 1 : (k + XBLK + 1) * W + 1],
            x_d[:, k * W : (k + XBLK) * W],
        )
        if ci == 3:
            nc.sync.dma_start(wcbd[:, :], wcbd_d)
            nc.sync.dma_start(bc2[:, :], bc_d)

    # dw output (lrelu'd, bf16) == rhs of the 1x1 conv
    dwout = dwp.tile([128, H * W], bf16)



    # ---------------- kernel-generating MLP ----------------
    hid_ps = po.tile([C, S], f32, tag="po")
    nc.tensor.matmul(hid_ps[:, :], lhsT=wk1t, rhs=dts, start=True, stop=True)
    hid_sb = const.tile([C, S], bf16)
    nc.scalar.activation(hid_sb[:, :], hid_ps[:, :], PRELU, alpha=0.1)

    kcols = const.tile([2 * C, KK * KK], f32)
    kps = po.tile([2 * C, KK * KK], f32, tag="po")
    for t in range(KK * KK):
        nc.tensor.matmul(
            kps[0:C, t : t + 1],
            lhsT=wk2td[:, t * 128 : t * 128 + C],
            rhs=hid_sb[:, 0:1],
            start=True, stop=True,
        )
        nc.tensor.matmul(
            kps[C : 2 * C, t : t + 1],
            lhsT=wk2td[:, t * 128 + C : (t + 1) * 128],
            rhs=hid_sb[:, 1:2],
            start=True, stop=True,
        )
    nc.vector.tensor_copy(kcols[:, :], kps[:, :])

    # per-tap diagonal weights for the PE region
    id_i = const.tile([128, 128], mybir.dt.int32)
    nc.gpsimd.iota(id_i[:, :], pattern=[[1, 128]], base=0, channel_multiplier=-1)
    idf = const.tile([128, 128], bf16)
    nc.vector.tensor_scalar(idf[:, :], id_i[:, :], 0, None, A.is_equal)
    diag = const.tile([128, KK * KK * 128], bf16)
    for t in range(KK * KK):
        nc.vector.tensor_scalar_mul(
            diag[:, t * 128 : (t + 1) * 128], idf[:, :], kcols[:, t : t + 1]
        )
    # negated diagonals for the W-edge wrap corrections (taps dj=0 and dj=2)
    negk = const.tile([2 * C, KK * KK], f32)
    nc.vector.tensor_scalar_mul(negk[:, :], kcols[:, :], -1.0)
    negdiag = const.tile([128, 6 * 128], bf16)
    for j, (di, dj) in enumerate(CORR):
        t = di * KK + dj
        nc.vector.tensor_scalar_mul(
            negdiag[:, j * 128 : (j + 1) * 128], idf[:, :], negk[:, t : t + 1]
        )

    for k in _xact:
        nc.scalar.dma_start(
            xs[:, (k + 1) * W + 1 : (k + XBLK + 1) * W + 1],
            x_d[:, k * W : (k + XBLK) * W],
        )

    xs3 = xs[:, 1 : 1 + XR * W].rearrange("p (r w) -> p r w", w=W)  # [128, XR, W]
    dw3 = dwout[:, :].rearrange("p (r w) -> p r w", w=W)

    # ---------------- region emitters ----------------
    def emit_A():
        """PE: 9 full-width taps + 6 strided W-edge corrections per 4-row
        bank; ACT Prelu evacuates to dwout."""
        banks = [r0 + 4 * i for r0, n in STRIPES_A for i in range(n // 4)]
        for rb in banks:
            if True:
                ps = pdw.tile([128, 512], f32, tag="pdw", name=f"pdw{rb}")
                bank = ps[:, :]
                for i, (di, dj) in enumerate(TAPS[:-1]):
                    t = di * KK + dj
                    base = (rb + di) * W + dj
                    nc.tensor.matmul(
                        bank[:, :],
                        lhsT=diag[:, t * 128 : (t + 1) * 128],
                        rhs=xs[:, base : base + 512],
                        start=(i == 0),
                        stop=False,
                    )
                for j, (di, dj) in enumerate(CORR):
                    if dj == 0:
                        dst = bank[:, 0 : 512 : W]
                        c0 = (rb + di) * W
                    else:
                        dst = bank[:, W - 1 : 512 : W]
                        c0 = (rb + di + 1) * W + 1
                    nc.tensor.matmul(
                        dst,
                        lhsT=negdiag[:, j * 128 : (j + 1) * 128],
                        rhs=xs[:, c0 : c0 + 3 * W + 1 : W],
                        start=False,
                        stop=False,
                    )
                di, dj = TAPS[-1]
                t = di * KK + dj
                base = (rb + di) * W + dj
                nc.tensor.matmul(
                    bank[:, :],
                    lhsT=diag[:, t * 128 : (t + 1) * 128],
                    rhs=xs[:, base : base + 512],
                    start=False,
                    stop=True,
                )
            nc.scalar.activation(
                dwout[:, rb * W : (rb + 4) * W], ps[:, :], PRELU, alpha=0.1
            )
            yield rb, 4

    def emit_B2():
        """DVE: scale-mul into tmp (4x), tensor-tensor add into dwout (2x);
        lrelu on ACT."""
        for ci, (r0, n) in enumerate(STRIPES_B2):
            last = ci == len(STRIPES_B2) - 1
            dst = dwout[:, r0 * W : (r0 + n) * W]
            t0 = 0 * KK + 1
            nc.vector.tensor_scalar_mul(
                dst, xs[:, r0 * W + 1 : (r0 + n) * W + 1], kcols[:, t0 : t0 + 1]
            )
            for di, dj in TAPS:
                if (di, dj) == (0, 1):
                    continue
                t = di * KK + dj
                w_lo, s_lo, ncol = _tap_geom(dj)
                tm = tmpp.tile([128, 16 * W], bf16, tag="b2t", name=f"tm{r0}_{t}")
                tm3 = tm[:, :].rearrange("p (r w) -> p r w", w=W)
                src = xs3[:, r0 + di : r0 + di + n, s_lo : s_lo + ncol]
                nc.vector.tensor_scalar_mul(tm3[:, 0:n, 0:ncol], src, kcols[:, t : t + 1])
                nc.vector.tensor_tensor(
                    dw3[:, r0 : r0 + n, w_lo : w_lo + ncol],
                    dw3[:, r0 : r0 + n, w_lo : w_lo + ncol],
                    tm3[:, 0:n, 0:ncol],
                    A.add,
                )
            nc.scalar.activation(dst, dst, PRELU, alpha=0.1)
            yield r0, n

    def emit_B3():
        """Pool: scale-mul into tmp (TensorScalarPtr) + tensor-tensor add
        into dwout; lrelu on ACT."""
        B3MAX = max(n for _, n in STRIPES_B3)
        for ci, (r0, n) in enumerate(STRIPES_B3):
            dst = dwout[:, r0 * W : (r0 + n) * W]
            t0 = 0 * KK + 1
            nc.gpsimd.tensor_scalar_mul(
                dst, xs[:, r0 * W + 1 : (r0 + n) * W + 1], kcols[:, t0 : t0 + 1]
            )
            for di, dj in TAPS:
                if (di, dj) == (0, 1):
                    continue
                t = di * KK + dj
                w_lo, s_lo, ncol = _tap_geom(dj)
                pt = sbp.tile([128, B3MAX * W], bf16, tag="p3t", name=f"pt{r0}_{t}")
                pt3 = pt[:, :].rearrange("p (r w) -> p r w", w=W)
                src = xs3[:, r0 + di : r0 + di + n, s_lo : s_lo + ncol]
                nc.gpsimd.tensor_scalar_mul(
                    pt3[:, 0:n, 0:ncol], src, kcols[:, t : t + 1]
                )
                nc.gpsimd.tensor_tensor(
                    dw3[:, r0 : r0 + n, w_lo : w_lo + ncol],
                    dw3[:, r0 : r0 + n, w_lo : w_lo + ncol],
                    pt3[:, 0:n, 0:ncol],
                    A.add,
                )
            nc.scalar.activation(dst, dst, PRELU, alpha=0.1)
            yield r0, n

    # ------- merged emission: dw chunks + 1x1 groups by virtual clocks -----
    def emit_group(g):
        rb = g * 8
        ps1 = po.tile([128, 1024], f32, tag="po", name=f"po{g}")
        for half in range(2):
            off = (rb + 4 * half) * W
            nc.tensor.matmul(
                ps1[:, half * 512 : (half + 1) * 512],
                lhsT=wcbd[:, :],
                rhs=dwout[:, off : off + 512],
                start=True, stop=True,
            )
        ob = obp.tile([128, 1024], bf16, tag="ob", name=f"ob{g}")
        emit_group.count += 1
        nc.scalar.activation(ob[:, :], ps1[:, :], IDENT, bias=bc2[:, 0:1])
        nc.sync.dma_start(out_d[:, rb * W : (rb + 8) * W], ob[:, :])

    emit_group.count = 0
    FILL = K_FILL  # rough ns before first dw work can start
    regions = [
        (emit_A(), PACE_A),
        (emit_B2(), PACE_B2),
        (emit_B3(), PACE_B3),
    ]
    clocks = [FILL] * len(regions)
    done_at: dict[int, float] = {}  # 4-row unit index -> est completion
    pending: list[tuple[float, int]] = []  # (ready est, 1x1 group)
    queued: set[int] = set()
    idx = list(range(len(regions)))
    while idx or pending:
        tnext = min((clocks[j] for j in idx), default=1e18)
        if pending and pending[0][0] <= tnext:
            _, g = pending.pop(0)
            emit_group(g)
            continue
        if not idx:
            continue
        i = min(idx, key=lambda j: clocks[j])
        ggen, pace = regions[i]
        try:
            r0, n = next(ggen)
        except StopIteration:
            idx.remove(i)
            continue
        assert r0 % 4 == 0 and n % 4 == 0
        clocks[i] += pace * n
        for u in range(r0 // 4, (r0 + n) // 4):
            done_at[u] = clocks[i]
        for g in range(H // 8):
            if g in queued:
                continue
            if 2 * g in done_at and 2 * g + 1 in done_at:
                queued.add(g)
                pending.append((max(done_at[2 * g], done_at[2 * g + 1]), g))
        pending.sort()
    assert len(queued) == H // 8, f"unemitted 1x1 groups: {sorted(set(range(16)) - queued)}"


# ---------------------------------------------------------------------------
# host-side entry point
# ---------------------------------------------------------------------------

_PROGRAM_CACHE: dict[str, bass.Bass] = {}


def _get_program() -> bass.Bass:
    if "p" not in _PROGRAM_CACHE:
        _PROGRAM_CACHE["p"] = build_program()
    return _PROGRAM_CACHE["p"]


def _host_prep(inputs: dict):
    import ml_dtypes

    x = np.asarray(inputs["x"], dtype=np.float32)
    d = np.asarray(inputs["d"], dtype=np.float32)
    Wk1 = np.asarray(inputs["Wk1"], dtype=np.float32)
    Wk2 = np.asarray(inputs["Wk2"], dtype=np.float32)
    Wc = np.asarray(inputs["Wc"], dtype=np.float32)
    bc = np.asarray(inputs["bc"], dtype=np.float32)

    wk1t = np.ascontiguousarray(Wk1.T)
    w = Wk2.reshape(C, KK * KK, C).transpose(2, 1, 0)  # (j, t, c)
    wk2td = np.ascontiguousarray(
        np.concatenate([w, w], axis=2).reshape(C, KK * KK * 2 * C)
    )
    wct = np.ascontiguousarray(Wc.T)
    wcbd = np.zeros((2 * C, 2 * C), np.float32)
    wcbd[0:C, 0:C] = wct
    wcbd[C:, C:] = wct
    wcbd = wcbd.astype(ml_dtypes.bfloat16)
    bc2 = np.ascontiguousarray(np.concatenate([bc, bc]).reshape(2 * C, 1))

    xcast = x.astype(ml_dtypes.bfloat16)

    in_maps = []
    for i in range(NCORES):
        xsamp = np.ascontiguousarray(xcast[S * i : S * (i + 1)].reshape(S * C, H * W))
        dT = np.ascontiguousarray(d[S * i : S * (i + 1)].T)
        mlpw = np.concatenate([dT, wk1t, wk2td], axis=1).astype(ml_dtypes.bfloat16)
        in_maps.append(
            {
                "x": xsamp,
                "mlpw": np.ascontiguousarray(mlpw),
                "wcbd": wcbd,
                "bc2": bc2,
            }
        )
    return in_maps


def run_on_hw(inputs: dict, **kwargs):
    from concourse.bass_utils import run_bass_kernel_spmd

    nc = _get_program()
    in_maps = _host_prep(inputs)
    res = run_bass_kernel_spmd(nc, in_maps, core_ids=list(range(NCORES)), **kwargs)
    outs = res.results
    B = S * NCORES
    out = np.empty((B, C, H, W), dtype=np.float32)
    for i in range(NCORES):
        out[S * i : S * (i + 1)] = (
            outs[i]["out"].astype(np.float32).reshape(S, C, H, W)
        )
    return out, res


def kernel(**inputs) -> np.ndarray:
    out, _ = run_on_hw(inputs)
    return out


if __name__ == "__main__":
    nc = build_program()
    print("program built OK")


# revision 6
# speedup vs baseline: 2.8377x; 1.0112x over previous
"""Trainium2 Bass kernel for nn_DA_conv: per-sample dynamic depthwise 3x3 conv
(+LeakyReLU) followed by a 1x1 pointwise conv, with the 3x3 kernels produced by
a small per-sample MLP.

Strategy (8 NeuronCores, batch data-parallel, 2 samples per core):
  - SBUF partition p = (sample s = p//64, channel c = p%64); the feature map
    is resident in bf16 with top/bottom halo rows plus a one-element pad on
    each end (no W padding: edge columns are handled by full-width taps plus
    strided correction matmuls / partial-width vector ops).
  - The image rows are split into three regions, one per compute engine, so
    the depthwise conv runs on PE, DVE and GPSIMD concurrently:
      A  : PE diagonal matmuls (9 full-width PSUM-accumulated taps + 6
           strided W-edge corrections per 4-row bank), ACT Prelu evac.
      B2 : DVE per-tap scale-muls (4x mode) + tensor-tensor adds.
      B3 : gpsimd scalar_tensor_tensor MAC chain.
    LeakyReLU for B2/B3 runs on ACT (Prelu), interleaved by estimated
    readiness so the in-order ACT queue never stalls the PSUM pipeline.
  - 1x1 conv: one 128x128 block-diagonal matmul per PSUM bank covers both
    samples at once; ACT evacuates with the bias via Identity+bias.
  - DMA: input + output both on the otherwise-idle SP queue; compute engines
    never issue DMAs.
"""

import sys

sys.path.insert(0, "/opt/trn_rl_repo")

from contextlib import ExitStack

import numpy as np

import concourse.bacc as bacc
import concourse.bass as bass
import concourse.mybir as mybir
import concourse.tile as tile

S = 2            # samples per core
C = 64           # channels
H = W = 128      # spatial
KK = 3           # conv kernel size
NCORES = 8
XR = H + 2       # resident rows incl halo

f32 = mybir.dt.float32
bf16 = mybir.dt.bfloat16
A = mybir.AluOpType
PRELU = mybir.ActivationFunctionType.Prelu
IDENT = mybir.ActivationFunctionType.Identity

# Row stripes per engine (cover 0..128, multiples of 4).
# Interleaved so 1x1-group completions stagger instead of bunching at the end.
STRIPES_A = [(0, 16), (28, 16), (56, 16), (84, 16), (124, 4)]   # PE, 68 rows
STRIPES_B2 = [(44, 12), (16, 12), (112, 12), (72, 4)]           # DVE, 40 rows
STRIPES_B3 = [(100, 12), (76, 8)]                               # Pool, 20 rows
R_A = sum(n for _, n in STRIPES_A)
R_B2 = sum(n for _, n in STRIPES_B2)
R_B3 = sum(n for _, n in STRIPES_B3)

# virtual-clock pace estimates (ns per image row) for emission ordering
PACE_A = 490.0
PACE_B2 = 890.0
PACE_B3 = 1850.0
K_FILL = 3000.0
K_XACT = [6, 11]     # consume-order chunk indices carried by the ACT queue
K_TAILSPLIT = 4      # last-N 1x1 groups: evac halves split across ACT/DVE
K_ASPLIT = 0

TAPS = [(di, dj) for di in range(KK) for dj in range(KK)]
CORR = [(di, dj) for dj in (0, 2) for di in range(KK)]


def _tap_geom(dj):
    """(out_col_lo, src_col_lo, ncols) for a horizontal tap shift."""
    if dj == 0:
        return 1, 0, W - 1
    if dj == 1:
        return 0, 0, W
    return 0, 1, W - 1


def build_program() -> bass.Bass:
    nc = bacc.Bacc("TRN2", target_bir_lowering=False, debug=False)

    x_d = nc.dram_tensor("x", [S * C, H * W], bf16, kind="ExternalInput").ap()
    # mlpw = [dT (2) | Wk1.T (64) | wk2td (1152)] in one tensor -> one DMA
    # (wk2td[j, t*128 + s*64 + c] = Wk2[c*9 + t, j], tap-major, sample-dup)
    mlpw_d = nc.dram_tensor(
        "mlpw", [C, S + C + KK * KK * 2 * C], bf16, kind="ExternalInput"
    ).ap()
    wcbd_d = nc.dram_tensor("wcbd", [2 * C, 2 * C], bf16, kind="ExternalInput").ap()
    bc_d = nc.dram_tensor("bc2", [2 * C, 1], f32, kind="ExternalInput").ap()
    out_d = nc.dram_tensor("out", [S * C, H * W], bf16, kind="ExternalOutput").ap()

    with tile.TileContext(nc) as tc, ExitStack() as ctx:
        _body(ctx, tc, x_d, mlpw_d, wcbd_d, bc_d, out_d)
    nc.compile()
    return nc


def _body(ctx, tc, x_d, mlpw_d, wcbd_d, bc_d, out_d):
    nc = tc.nc
    const = ctx.enter_context(tc.tile_pool(name="const", bufs=1))
    xpool = ctx.enter_context(tc.tile_pool(name="xs", bufs=1))
    dwp = ctx.enter_context(tc.tile_pool(name="dw", bufs=1))
    tmpp = ctx.enter_context(tc.tile_pool(name="tmp", bufs=2))
    sbp = ctx.enter_context(tc.tile_pool(name="sb", bufs=2))
    obp = ctx.enter_context(tc.tile_pool(name="ob", bufs=3))
    pdw = ctx.enter_context(tc.tile_pool(name="pdw", bufs=2, space="PSUM"))
    po = ctx.enter_context(tc.tile_pool(name="po", bufs=3, space="PSUM"))

    # ---------------- const loads (qSP, before x) ----------------
    mlpw = const.tile([C, S + C + KK * KK * 2 * C], bf16)
    nc.sync.dma_start(mlpw[:, :], mlpw_d)
    dts = mlpw[:, 0:S]
    wk1t = mlpw[:, S : S + C]
    wk2td = mlpw[:, S + C : S + C + KK * KK * 2 * C]

    # ---------------- resident x (halo rows zeroed, +-1 elem pad) ----------
    xs = xpool.tile([128, XR * W + 2], bf16)
    nc.vector.memset(xs[:, 0 : W + 1], 0.0)
    nc.vector.memset(xs[:, (XR - 1) * W + 1 : XR * W + 2], 0.0)
    XBLK = 8

    def _xchunks():
        """order the 16 aligned 8-row x chunks by estimated consume time."""
        consume_at = {}
        for stripes, pace in ((STRIPES_A, PACE_A), (STRIPES_B2, PACE_B2),
                              (STRIPES_B3, PACE_B3)):
            t = 0.0
            for r0, n in stripes:
                for r in range(r0, r0 + n):
                    consume_at[r] = t + pace * (r - r0)
                t += pace * n

        def consume(k):
            rows = range(max(0, k - 1), min(H, k + XBLK + 1))
            return min(consume_at[r] for r in rows)

        return sorted(range(0, H, XBLK), key=consume)

    wcbd = const.tile([2 * C, 2 * C], bf16)
    bc2 = const.tile([2 * C, 1], f32)
    _xorder = _xchunks()
    # SP carries the early-consumed chunks; ACT takes K_XACT positions after
    # the MLP section (so the in-order ACT queue does the MLP prelu first).
    _xact = [k for ci, k in enumerate(_xorder) if ci in K_XACT]
    _xsp = [k for k in _xorder if k not in _xact]
    for ci, k in enumerate(_xsp):
        nc.sync.dma_start(
            xs[:, (k + 1) * W + 1 : (k + XBLK + 1) * W + 1],
            x_d[:, k * W : (k + XBLK) * W],
        )
        if ci == 3:
            nc.sync.dma_start(wcbd[:, :], wcbd_d)
            nc.sync.dma_start(bc2[:, :], bc_d)

    # dw output (lrelu'd, bf16) == rhs of the 1x1 conv
    dwout = dwp.tile([128, H * W], bf16)



    # ---------------- kernel-generating MLP ----------------
    hid_ps = po.tile([C, S], f32, tag="po")
    nc.tensor.matmul(hid_ps[:, :], lhsT=wk1t, rhs=dts, start=True, stop=True)
    hid_sb = const.tile([C, S], bf16)
    nc.scalar.activation(hid_sb[:, :], hid_ps[:, :], PRELU, alpha=0.1)

    kcols = const.tile([2 * C, KK * KK], f32)
    kps = po.tile([2 * C, KK * KK], f32, tag="po")
    for t in range(KK * KK):
        nc.tensor.matmul(
            kps[0:C, t : t + 1],
            lhsT=wk2td[:, t * 128 : t * 128 + C],
            rhs=hid_sb[:, 0:1],
            start=True, stop=True,
        )
        nc.tensor.matmul(
            kps[C : 2 * C, t : t + 1],
            lhsT=wk2td[:, t * 128 + C : (t + 1) * 128],
            rhs=hid_sb[:, 1:2],
            start=True, stop=True,
        )
    nc.vector.tensor_copy(kcols[:, :], kps[:, :])

    # per-tap diagonal weights for the PE region
    id_i = const.tile([128, 128], mybir.dt.int32)
    nc.gpsimd.iota(id_i[:, :], pattern=[[1, 128]], base=0, channel_multiplier=-1)
    idf = const.tile([128, 128], bf16)
    nc.vector.tensor_scalar(idf[:, :], id_i[:, :], 0, None, A.is_equal)
    diag = const.tile([128, KK * KK * 128], bf16)
    for t in range(KK * KK):
        nc.vector.tensor_scalar_mul(
            diag[:, t * 128 : (t + 1) * 128], idf[:, :], kcols[:, t : t + 1]
        )
    # negated diagonals for the W-edge wrap corrections (taps dj=0 and dj=2)
    negk = const.tile([2 * C, KK * KK], f32)
    nc.vector.tensor_scalar_mul(negk[:, :], kcols[:, :], -1.0)
    negdiag = const.tile([128, 6 * 128], bf16)
    for j, (di, dj) in enumerate(CORR):
        t = di * KK + dj
        nc.vector.tensor_scalar_mul(
            negdiag[:, j * 128 : (j + 1) * 128], idf[:, :], negk[:, t : t + 1]
        )

    for k in _xact:
        nc.scalar.dma_start(
            xs[:, (k + 1) * W + 1 : (k + XBLK + 1) * W + 1],
            x_d[:, k * W : (k + XBLK) * W],
        )

    xs3 = xs[:, 1 : 1 + XR * W].rearrange("p (r w) -> p r w", w=W)  # [128, XR, W]
    dw3 = dwout[:, :].rearrange("p (r w) -> p r w", w=W)

    # ---------------- region emitters ----------------
    def emit_A():
        """PE: 9 full-width taps + 6 strided W-edge corrections per 4-row
        bank; ACT Prelu evacuates to dwout."""
        banks = [r0 + 4 * i for r0, n in STRIPES_A for i in range(n // 4)]
        emit_A.bi = 0
        emit_A.nb = len(banks)
        for rb in banks:
            if True:
                ps = pdw.tile([128, 512], f32, tag="pdw", name=f"pdw{rb}")
                bank = ps[:, :]
                for i, (di, dj) in enumerate(TAPS[:-1]):
                    t = di * KK + dj
                    base = (rb + di) * W + dj
                    nc.tensor.matmul(
                        bank[:, :],
                        lhsT=diag[:, t * 128 : (t + 1) * 128],
                        rhs=xs[:, base : base + 512],
                        start=(i == 0),
                        stop=False,
                    )
                for j, (di, dj) in enumerate(CORR):
                    if dj == 0:
                        dst = bank[:, 0 : 512 : W]
                        c0 = (rb + di) * W
                    else:
                        dst = bank[:, W - 1 : 512 : W]
                        c0 = (rb + di + 1) * W + 1
                    nc.tensor.matmul(
                        dst,
                        lhsT=negdiag[:, j * 128 : (j + 1) * 128],
                        rhs=xs[:, c0 : c0 + 3 * W + 1 : W],
                        start=False,
                        stop=False,
                    )
                di, dj = TAPS[-1]
                t = di * KK + dj
                base = (rb + di) * W + dj
                nc.tensor.matmul(
                    bank[:, :],
                    lhsT=diag[:, t * 128 : (t + 1) * 128],
                    rhs=xs[:, base : base + 512],
                    start=False,
                    stop=True,
                )
            emit_A.bi += 1
            if K_ASPLIT and emit_A.bi > emit_A.nb - K_ASPLIT:
                nc.scalar.activation(
                    dwout[:, rb * W : rb * W + 256], ps[:, 0:256], PRELU, alpha=0.1
                )
                nc.vector.scalar_tensor_tensor(
                    dwout[:, rb * W + 256 : (rb + 4) * W], ps[:, 256:512], 0.1,
                    ps[:, 256:512], op0=A.mult, op1=A.max,
                )
            else:
                nc.scalar.activation(
                    dwout[:, rb * W : (rb + 4) * W], ps[:, :], PRELU, alpha=0.1
                )
            yield rb, 4

    def emit_B2():
        """DVE: scale-mul into tmp (4x), tensor-tensor add into dwout (2x);
        lrelu on ACT."""
        for ci, (r0, n) in enumerate(STRIPES_B2):
            last = ci == len(STRIPES_B2) - 1
            dst = dwout[:, r0 * W : (r0 + n) * W]
            t0 = 0 * KK + 1
            nc.vector.tensor_scalar_mul(
                dst, xs[:, r0 * W + 1 : (r0 + n) * W + 1], kcols[:, t0 : t0 + 1]
            )
            for di, dj in TAPS:
                if (di, dj) == (0, 1):
                    continue
                t = di * KK + dj
                w_lo, s_lo, ncol = _tap_geom(dj)
                tm = tmpp.tile([128, 16 * W], bf16, tag="b2t", name=f"tm{r0}_{t}")
                tm3 = tm[:, :].rearrange("p (r w) -> p r w", w=W)
                src = xs3[:, r0 + di : r0 + di + n, s_lo : s_lo + ncol]
                nc.vector.tensor_scalar_mul(tm3[:, 0:n, 0:ncol], src, kcols[:, t : t + 1])
                nc.vector.tensor_tensor(
                    dw3[:, r0 : r0 + n, w_lo : w_lo + ncol],
                    dw3[:, r0 : r0 + n, w_lo : w_lo + ncol],
                    tm3[:, 0:n, 0:ncol],
                    A.add,
                )
            nc.scalar.activation(dst, dst, PRELU, alpha=0.1)
            yield r0, n

    def emit_B3():
        """Pool: scale-mul into tmp (TensorScalarPtr) + tensor-tensor add
        into dwout; lrelu on ACT."""
        B3MAX = max(n for _, n in STRIPES_B3)
        for ci, (r0, n) in enumerate(STRIPES_B3):
            dst = dwout[:, r0 * W : (r0 + n) * W]
            t0 = 0 * KK + 1
            nc.gpsimd.tensor_scalar_mul(
                dst, xs[:, r0 * W + 1 : (r0 + n) * W + 1], kcols[:, t0 : t0 + 1]
            )
            for di, dj in TAPS:
                if (di, dj) == (0, 1):
                    continue
                t = di * KK + dj
                w_lo, s_lo, ncol = _tap_geom(dj)
                pt = sbp.tile([128, B3MAX * W], bf16, tag="p3t", name=f"pt{r0}_{t}")
                pt3 = pt[:, :].rearrange("p (r w) -> p r w", w=W)
                src = xs3[:, r0 + di : r0 + di + n, s_lo : s_lo + ncol]
                nc.gpsimd.tensor_scalar_mul(
                    pt3[:, 0:n, 0:ncol], src, kcols[:, t : t + 1]
                )
                nc.gpsimd.tensor_tensor(
                    dw3[:, r0 : r0 + n, w_lo : w_lo + ncol],
                    dw3[:, r0 : r0 + n, w_lo : w_lo + ncol],
                    pt3[:, 0:n, 0:ncol],
                    A.add,
                )
            nc.scalar.activation(dst, dst, PRELU, alpha=0.1)
            yield r0, n

    # ------- merged emission: dw chunks + 1x1 groups by virtual clocks -----
    def emit_group(g):
        rb = g * 8
        ps1 = po.tile([128, 1024], f32, tag="po", name=f"po{g}")
        for half in range(2):
            off = (rb + 4 * half) * W
            nc.tensor.matmul(
                ps1[:, half * 512 : (half + 1) * 512],
                lhsT=wcbd[:, :],
                rhs=dwout[:, off : off + 512],
                start=True, stop=True,
            )
        ob = obp.tile([128, 1024], bf16, tag="ob", name=f"ob{g}")
        emit_group.count += 1
        if K_TAILSPLIT and emit_group.count > H // 8 - K_TAILSPLIT:
            # split the evac: ACT and DVE each do one 512 half concurrently
            nc.scalar.activation(ob[:, 0:512], ps1[:, 0:512], IDENT, bias=bc2[:, 0:1])
            nc.vector.tensor_scalar_add(ob[:, 512:1024], ps1[:, 512:1024], bc2[:, 0:1])
        else:
            nc.scalar.activation(ob[:, :], ps1[:, :], IDENT, bias=bc2[:, 0:1])
        nc.sync.dma_start(out_d[:, rb * W : (rb + 8) * W], ob[:, :])

    emit_group.count = 0
    FILL = K_FILL  # rough ns before first dw work can start
    regions = [
        (emit_A(), PACE_A),
        (emit_B2(), PACE_B2),
        (emit_B3(), PACE_B3),
    ]
    clocks = [FILL] * len(regions)
    done_at: dict[int, float] = {}  # 4-row unit index -> est completion
    pending: list[tuple[float, int]] = []  # (ready est, 1x1 group)
    queued: set[int] = set()
    idx = list(range(len(regions)))
    while idx or pending:
        tnext = min((clocks[j] for j in idx), default=1e18)
        if pending and pending[0][0] <= tnext:
            _, g = pending.pop(0)
            emit_group(g)
            continue
        if not idx:
            continue
        i = min(idx, key=lambda j: clocks[j])
        ggen, pace = regions[i]
        try:
            r0, n = next(ggen)
        except StopIteration:
            idx.remove(i)
            continue
        assert r0 % 4 == 0 and n % 4 == 0
        clocks[i] += pace * n
        for u in range(r0 // 4, (r0 + n) // 4):
            done_at[u] = clocks[i]
        for g in range(H // 8):
            if g in queued:
                continue
            if 2 * g in done_at and 2 * g + 1 in done_at:
                queued.add(g)
                pending.append((max(done_at[2 * g], done_at[2 * g + 1]), g))
        pending.sort()
    assert len(queued) == H // 8, f"unemitted 1x1 groups: {sorted(set(range(16)) - queued)}"


# ---------------------------------------------------------------------------
# host-side entry point
# ---------------------------------------------------------------------------

_PROGRAM_CACHE: dict[str, bass.Bass] = {}


def _get_program() -> bass.Bass:
    if "p" not in _PROGRAM_CACHE:
        _PROGRAM_CACHE["p"] = build_program()
    return _PROGRAM_CACHE["p"]


def _host_prep(inputs: dict):
    import ml_dtypes

    x = np.asarray(inputs["x"], dtype=np.float32)
    d = np.asarray(inputs["d"], dtype=np.float32)
    Wk1 = np.asarray(inputs["Wk1"], dtype=np.float32)
    Wk2 = np.asarray(inputs["Wk2"], dtype=np.float32)
    Wc = np.asarray(inputs["Wc"], dtype=np.float32)
    bc = np.asarray(inputs["bc"], dtype=np.float32)

    wk1t = np.ascontiguousarray(Wk1.T)
    w = Wk2.reshape(C, KK * KK, C).transpose(2, 1, 0)  # (j, t, c)
    wk2td = np.ascontiguousarray(
        np.concatenate([w, w], axis=2).reshape(C, KK * KK * 2 * C)
    )
    wct = np.ascontiguousarray(Wc.T)
    wcbd = np.zeros((2 * C, 2 * C), np.float32)
    wcbd[0:C, 0:C] = wct
    wcbd[C:, C:] = wct
    wcbd = wcbd.astype(ml_dtypes.bfloat16)
    bc2 = np.ascontiguousarray(np.concatenate([bc, bc]).reshape(2 * C, 1))

    xcast = x.astype(ml_dtypes.bfloat16)

    in_maps = []
    for i in range(NCORES):
        xsamp = np.ascontiguousarray(xcast[S * i : S * (i + 1)].reshape(S * C, H * W))
        dT = np.ascontiguousarray(d[S * i : S * (i + 1)].T)
        mlpw = np.concatenate([dT, wk1t, wk2td], axis=1).astype(ml_dtypes.bfloat16)
        in_maps.append(
            {
                "x": xsamp,
                "mlpw": np.ascontiguousarray(mlpw),
                "wcbd": wcbd,
                "bc2": bc2,
            }
        )
    return in_maps


def run_on_hw(inputs: dict, **kwargs):
    from concourse.bass_utils import run_bass_kernel_spmd

    nc = _get_program()
    in_maps = _host_prep(inputs)
    res = run_bass_kernel_spmd(nc, in_maps, core_ids=list(range(NCORES)), **kwargs)
    outs = res.results
    B = S * NCORES
    out = np.empty((B, C, H, W), dtype=np.float32)
    for i in range(NCORES):
        out[S * i : S * (i + 1)] = (
            outs[i]["out"].astype(np.float32).reshape(S, C, H, W)
        )
    return out, res


def kernel(**inputs) -> np.ndarray:
    out, _ = run_on_hw(inputs)
    return out


if __name__ == "__main__":
    nc = build_program()
    print("program built OK")


# revision 7
# speedup vs baseline: 2.8724x; 1.0123x over previous
"""Trainium2 Bass kernel for nn_DA_conv: per-sample dynamic depthwise 3x3 conv
(+LeakyReLU) followed by a 1x1 pointwise conv, with the 3x3 kernels produced by
a small per-sample MLP.

Strategy (8 NeuronCores, batch data-parallel, 2 samples per core):
  - SBUF partition p = (sample s = p//64, channel c = p%64); the feature map
    is resident in bf16 with top/bottom halo rows plus a one-element pad on
    each end (no W padding: edge columns are handled by full-width taps plus
    strided correction matmuls / partial-width vector ops).
  - The image rows are split into three regions, one per compute engine, so
    the depthwise conv runs on PE, DVE and GPSIMD concurrently:
      A  : PE diagonal matmuls (9 full-width PSUM-accumulated taps + 6
           strided W-edge corrections per 4-row bank), ACT Prelu evac.
      B2 : DVE per-tap scale-muls (4x mode) + tensor-tensor adds.
      B3 : gpsimd scalar_tensor_tensor MAC chain.
    LeakyReLU for B2/B3 runs on ACT (Prelu), interleaved by estimated
    readiness so the in-order ACT queue never stalls the PSUM pipeline.
  - 1x1 conv: one 128x128 block-diagonal matmul per PSUM bank covers both
    samples at once; ACT evacuates with the bias via Identity+bias.
  - DMA: input + output both on the otherwise-idle SP queue; compute engines
    never issue DMAs.
"""

import sys

sys.path.insert(0, "/opt/trn_rl_repo")

from contextlib import ExitStack

import numpy as np

import concourse.bacc as bacc
import concourse.bass as bass
import concourse.mybir as mybir
import concourse.tile as tile

S = 2            # samples per core
C = 64           # channels
H = W = 128      # spatial
KK = 3           # conv kernel size
NCORES = 8
XR = H + 2       # resident rows incl halo

f32 = mybir.dt.float32
bf16 = mybir.dt.bfloat16
A = mybir.AluOpType
PRELU = mybir.ActivationFunctionType.Prelu
IDENT = mybir.ActivationFunctionType.Identity

# Row stripes per engine (cover 0..128, multiples of 4).
# Interleaved so 1x1-group completions stagger instead of bunching at the end.
STRIPES_A = [(0, 16), (28, 16), (56, 16), (84, 16), (124, 4)]   # PE, 68 rows
STRIPES_B2 = [(44, 12), (16, 12), (112, 12), (72, 4)]           # DVE, 40 rows
STRIPES_B3 = [(100, 12), (76, 8)]                               # Pool, 20 rows
R_A = sum(n for _, n in STRIPES_A)
R_B2 = sum(n for _, n in STRIPES_B2)
R_B3 = sum(n for _, n in STRIPES_B3)

# virtual-clock pace estimates (ns per image row) for emission ordering
PACE_A = 490.0
PACE_B2 = 890.0
PACE_B3 = 1850.0
K_FILL = 3000.0
K_XACT = [6, 11]     # consume-order chunk indices carried by the ACT queue
K_TAILSPLIT = 4      # last-N 1x1 groups: evac halves split across ACT/DVE
K_ASPLIT = 0
K_LSPLIT = 1  # last B2/B3 chunk: lrelu halves split across ACT/DVE

def _lrelu_maybe_split(nc, dst, n, split):
    if split:
        half = (n * W) // 2
        nc.scalar.activation(dst[:, 0:half], dst[:, 0:half],
                             mybir.ActivationFunctionType.Prelu, alpha=0.1)
        nc.vector.scalar_tensor_tensor(
            dst[:, half:], dst[:, half:], 0.1, dst[:, half:],
            op0=A.mult, op1=A.max)
    else:
        nc.scalar.activation(dst, dst, mybir.ActivationFunctionType.Prelu, alpha=0.1)

TAPS = [(di, dj) for di in range(KK) for dj in range(KK)]
CORR = [(di, dj) for dj in (0, 2) for di in range(KK)]


def _tap_geom(dj):
    """(out_col_lo, src_col_lo, ncols) for a horizontal tap shift."""
    if dj == 0:
        return 1, 0, W - 1
    if dj == 1:
        return 0, 0, W
    return 0, 1, W - 1


def build_program() -> bass.Bass:
    nc = bacc.Bacc("TRN2", target_bir_lowering=False, debug=False)

    x_d = nc.dram_tensor("x", [S * C, H * W], bf16, kind="ExternalInput").ap()
    # mlpw = [dT (2) | Wk1.T (64) | wk2td (1152)] in one tensor -> one DMA
    # (wk2td[j, t*128 + s*64 + c] = Wk2[c*9 + t, j], tap-major, sample-dup)
    mlpw_d = nc.dram_tensor(
        "mlpw", [C, S + C + KK * KK * 2 * C], bf16, kind="ExternalInput"
    ).ap()
    wcbd_d = nc.dram_tensor("wcbd", [2 * C, 2 * C], bf16, kind="ExternalInput").ap()
    bc_d = nc.dram_tensor("bc2", [2 * C, 1], f32, kind="ExternalInput").ap()
    out_d = nc.dram_tensor("out", [S * C, H * W], bf16, kind="ExternalOutput").ap()

    with tile.TileContext(nc) as tc, ExitStack() as ctx:
        _body(ctx, tc, x_d, mlpw_d, wcbd_d, bc_d, out_d)
    nc.compile()
    return nc


def _body(ctx, tc, x_d, mlpw_d, wcbd_d, bc_d, out_d):
    nc = tc.nc
    const = ctx.enter_context(tc.tile_pool(name="const", bufs=1))
    xpool = ctx.enter_context(tc.tile_pool(name="xs", bufs=1))
    dwp = ctx.enter_context(tc.tile_pool(name="dw", bufs=1))
    tmpp = ctx.enter_context(tc.tile_pool(name="tmp", bufs=2))
    sbp = ctx.enter_context(tc.tile_pool(name="sb", bufs=2))
    obp = ctx.enter_context(tc.tile_pool(name="ob", bufs=3))
    pdw = ctx.enter_context(tc.tile_pool(name="pdw", bufs=2, space="PSUM"))
    po = ctx.enter_context(tc.tile_pool(name="po", bufs=3, space="PSUM"))

    # ---------------- const loads (qSP, before x) ----------------
    mlpw = const.tile([C, S + C + KK * KK * 2 * C], bf16)
    nc.sync.dma_start(mlpw[:, :], mlpw_d)
    dts = mlpw[:, 0:S]
    wk1t = mlpw[:, S : S + C]
    wk2td = mlpw[:, S + C : S + C + KK * KK * 2 * C]

    # ---------------- resident x (halo rows zeroed, +-1 elem pad) ----------
    xs = xpool.tile([128, XR * W + 2], bf16)
    nc.vector.memset(xs[:, 0 : W + 1], 0.0)
    nc.vector.memset(xs[:, (XR - 1) * W + 1 : XR * W + 2], 0.0)
    XBLK = 8

    def _xchunks():
        """order the 16 aligned 8-row x chunks by estimated consume time."""
        consume_at = {}
        for stripes, pace in ((STRIPES_A, PACE_A), (STRIPES_B2, PACE_B2),
                              (STRIPES_B3, PACE_B3)):
            t = 0.0
            for r0, n in stripes:
                for r in range(r0, r0 + n):
                    consume_at[r] = t + pace * (r - r0)
                t += pace * n

        def consume(k):
            rows = range(max(0, k - 1), min(H, k + XBLK + 1))
            return min(consume_at[r] for r in rows)

        return sorted(range(0, H, XBLK), key=consume)

    wcbd = const.tile([2 * C, 2 * C], bf16)
    bc2 = const.tile([2 * C, 1], f32)
    _xorder = _xchunks()
    # SP carries the early-consumed chunks; ACT takes K_XACT positions after
    # the MLP section (so the in-order ACT queue does the MLP prelu first).
    _xact = [k for ci, k in enumerate(_xorder) if ci in K_XACT]
    _xsp = [k for k in _xorder if k not in _xact]
    for ci, k in enumerate(_xsp):
        nc.sync.dma_start(
            xs[:, (k + 1) * W + 1 : (k + XBLK + 1) * W + 1],
            x_d[:, k * W : (k + XBLK) * W],
        )
        if ci == 3:
            nc.sync.dma_start(wcbd[:, :], wcbd_d)
            nc.sync.dma_start(bc2[:, :], bc_d)

    # dw output (lrelu'd, bf16) == rhs of the 1x1 conv
    dwout = dwp.tile([128, H * W], bf16)



    # ---------------- kernel-generating MLP ----------------
    hid_ps = po.tile([C, S], f32, tag="po")
    nc.tensor.matmul(hid_ps[:, :], lhsT=wk1t, rhs=dts, start=True, stop=True)
    hid_sb = const.tile([C, S], bf16)
    nc.scalar.activation(hid_sb[:, :], hid_ps[:, :], PRELU, alpha=0.1)

    kcols = const.tile([2 * C, KK * KK], f32)
    kps = po.tile([2 * C, KK * KK], f32, tag="po")
    for t in range(KK * KK):
        nc.tensor.matmul(
            kps[0:C, t : t + 1],
            lhsT=wk2td[:, t * 128 : t * 128 + C],
            rhs=hid_sb[:, 0:1],
            start=True, stop=True,
        )
        nc.tensor.matmul(
            kps[C : 2 * C, t : t + 1],
            lhsT=wk2td[:, t * 128 + C : (t + 1) * 128],
            rhs=hid_sb[:, 1:2],
            start=True, stop=True,
        )
    nc.vector.tensor_copy(kcols[:, :], kps[:, :])

    # per-tap diagonal weights for the PE region
    id_i = const.tile([128, 128], mybir.dt.int32)
    nc.gpsimd.iota(id_i[:, :], pattern=[[1, 128]], base=0, channel_multiplier=-1)
    idf = const.tile([128, 128], bf16)
    nc.vector.tensor_scalar(idf[:, :], id_i[:, :], 0, None, A.is_equal)
    diag = const.tile([128, KK * KK * 128], bf16)
    for t in range(KK * KK):
        nc.vector.tensor_scalar_mul(
            diag[:, t * 128 : (t + 1) * 128], idf[:, :], kcols[:, t : t + 1]
        )
    # negated diagonals for the W-edge wrap corrections (taps dj=0 and dj=2)
    negk = const.tile([2 * C, KK * KK], f32)
    nc.vector.tensor_scalar_mul(negk[:, :], kcols[:, :], -1.0)
    negdiag = const.tile([128, 6 * 128], bf16)
    for j, (di, dj) in enumerate(CORR):
        t = di * KK + dj
        nc.vector.tensor_scalar_mul(
            negdiag[:, j * 128 : (j + 1) * 128], idf[:, :], negk[:, t : t + 1]
        )

    for k in _xact:
        nc.scalar.dma_start(
            xs[:, (k + 1) * W + 1 : (k + XBLK + 1) * W + 1],
            x_d[:, k * W : (k + XBLK) * W],
        )

    xs3 = xs[:, 1 : 1 + XR * W].rearrange("p (r w) -> p r w", w=W)  # [128, XR, W]
    dw3 = dwout[:, :].rearrange("p (r w) -> p r w", w=W)

    # ---------------- region emitters ----------------
    def emit_A():
        """PE: 9 full-width taps + 6 strided W-edge corrections per 4-row
        bank; ACT Prelu evacuates to dwout."""
        banks = [r0 + 4 * i for r0, n in STRIPES_A for i in range(n // 4)]
        emit_A.bi = 0
        emit_A.nb = len(banks)
        for rb in banks:
            if True:
                ps = pdw.tile([128, 512], f32, tag="pdw", name=f"pdw{rb}")
                bank = ps[:, :]
                for i, (di, dj) in enumerate(TAPS[:-1]):
                    t = di * KK + dj
                    base = (rb + di) * W + dj
                    nc.tensor.matmul(
                        bank[:, :],
                        lhsT=diag[:, t * 128 : (t + 1) * 128],
                        rhs=xs[:, base : base + 512],
                        start=(i == 0),
                        stop=False,
                    )
                for j, (di, dj) in enumerate(CORR):
                    if dj == 0:
                        dst = bank[:, 0 : 512 : W]
                        c0 = (rb + di) * W
                    else:
                        dst = bank[:, W - 1 : 512 : W]
                        c0 = (rb + di + 1) * W + 1
                    nc.tensor.matmul(
                        dst,
                        lhsT=negdiag[:, j * 128 : (j + 1) * 128],
                        rhs=xs[:, c0 : c0 + 3 * W + 1 : W],
                        start=False,
                        stop=False,
                    )
                di, dj = TAPS[-1]
                t = di * KK + dj
                base = (rb + di) * W + dj
                nc.tensor.matmul(
                    bank[:, :],
                    lhsT=diag[:, t * 128 : (t + 1) * 128],
                    rhs=xs[:, base : base + 512],
                    start=False,
                    stop=True,
                )
            emit_A.bi += 1
            if K_ASPLIT and emit_A.bi > emit_A.nb - K_ASPLIT:
                nc.scalar.activation(
                    dwout[:, rb * W : rb * W + 256], ps[:, 0:256], PRELU, alpha=0.1
                )
                nc.vector.scalar_tensor_tensor(
                    dwout[:, rb * W + 256 : (rb + 4) * W], ps[:, 256:512], 0.1,
                    ps[:, 256:512], op0=A.mult, op1=A.max,
                )
            else:
                nc.scalar.activation(
                    dwout[:, rb * W : (rb + 4) * W], ps[:, :], PRELU, alpha=0.1
                )
            yield rb, 4

    def emit_B2():
        """DVE: scale-mul into tmp (4x), tensor-tensor add into dwout (2x);
        lrelu on ACT."""
        for ci, (r0, n) in enumerate(STRIPES_B2):
            last = ci == len(STRIPES_B2) - 1
            dst = dwout[:, r0 * W : (r0 + n) * W]
            t0 = 0 * KK + 1
            nc.vector.tensor_scalar_mul(
                dst, xs[:, r0 * W + 1 : (r0 + n) * W + 1], kcols[:, t0 : t0 + 1]
            )
            for di, dj in TAPS:
                if (di, dj) == (0, 1):
                    continue
                t = di * KK + dj
                w_lo, s_lo, ncol = _tap_geom(dj)
                tm = tmpp.tile([128, 16 * W], bf16, tag="b2t", name=f"tm{r0}_{t}")
                tm3 = tm[:, :].rearrange("p (r w) -> p r w", w=W)
                src = xs3[:, r0 + di : r0 + di + n, s_lo : s_lo + ncol]
                nc.vector.tensor_scalar_mul(tm3[:, 0:n, 0:ncol], src, kcols[:, t : t + 1])
                nc.vector.tensor_tensor(
                    dw3[:, r0 : r0 + n, w_lo : w_lo + ncol],
                    dw3[:, r0 : r0 + n, w_lo : w_lo + ncol],
                    tm3[:, 0:n, 0:ncol],
                    A.add,
                )
            _lrelu_maybe_split(nc, dst, n,
                               K_LSPLIT and ci >= len(STRIPES_B2) - K_LSPLIT)
            yield r0, n

    def emit_B3():
        """Pool: scale-mul into tmp (TensorScalarPtr) + tensor-tensor add
        into dwout; lrelu on ACT."""
        B3MAX = max(n for _, n in STRIPES_B3)
        for ci, (r0, n) in enumerate(STRIPES_B3):
            dst = dwout[:, r0 * W : (r0 + n) * W]
            t0 = 0 * KK + 1
            nc.gpsimd.tensor_scalar_mul(
                dst, xs[:, r0 * W + 1 : (r0 + n) * W + 1], kcols[:, t0 : t0 + 1]
            )
            for di, dj in TAPS:
                if (di, dj) == (0, 1):
                    continue
                t = di * KK + dj
                w_lo, s_lo, ncol = _tap_geom(dj)
                pt = sbp.tile([128, B3MAX * W], bf16, tag="p3t", name=f"pt{r0}_{t}")
                pt3 = pt[:, :].rearrange("p (r w) -> p r w", w=W)
                src = xs3[:, r0 + di : r0 + di + n, s_lo : s_lo + ncol]
                nc.gpsimd.tensor_scalar_mul(
                    pt3[:, 0:n, 0:ncol], src, kcols[:, t : t + 1]
                )
                nc.gpsimd.tensor_tensor(
                    dw3[:, r0 : r0 + n, w_lo : w_lo + ncol],
                    dw3[:, r0 : r0 + n, w_lo : w_lo + ncol],
                    pt3[:, 0:n, 0:ncol],
                    A.add,
                )
            _lrelu_maybe_split(nc, dst, n,
                               K_LSPLIT and ci >= len(STRIPES_B3) - K_LSPLIT)
            yield r0, n

    # ------- merged emission: dw chunks + 1x1 groups by virtual clocks -----
    def emit_group(g):
        rb = g * 8
        ps1 = po.tile([128, 1024], f32, tag="po", name=f"po{g}")
        for half in range(2):
            off = (rb + 4 * half) * W
            nc.tensor.matmul(
                ps1[:, half * 512 : (half + 1) * 512],
                lhsT=wcbd[:, :],
                rhs=dwout[:, off : off + 512],
                start=True, stop=True,
            )
        ob = obp.tile([128, 1024], bf16, tag="ob", name=f"ob{g}")
        emit_group.count += 1
        if K_TAILSPLIT and emit_group.count > H // 8 - K_TAILSPLIT:
            # split the evac: ACT and DVE each do one 512 half concurrently
            nc.scalar.activation(ob[:, 0:512], ps1[:, 0:512], IDENT, bias=bc2[:, 0:1])
            nc.vector.tensor_scalar_add(ob[:, 512:1024], ps1[:, 512:1024], bc2[:, 0:1])
        else:
            nc.scalar.activation(ob[:, :], ps1[:, :], IDENT, bias=bc2[:, 0:1])
        nc.sync.dma_start(out_d[:, rb * W : (rb + 8) * W], ob[:, :])

    emit_group.count = 0
    FILL = K_FILL  # rough ns before first dw work can start
    regions = [
        (emit_A(), PACE_A),
        (emit_B2(), PACE_B2),
        (emit_B3(), PACE_B3),
    ]
    clocks = [FILL] * len(regions)
    done_at: dict[int, float] = {}  # 4-row unit index -> est completion
    pending: list[tuple[float, int]] = []  # (ready est, 1x1 group)
    queued: set[int] = set()
    idx = list(range(len(regions)))
    while idx or pending:
        tnext = min((clocks[j] for j in idx), default=1e18)
        if pending and pending[0][0] <= tnext:
            _, g = pending.pop(0)
            emit_group(g)
            continue
        if not idx:
            continue
        i = min(idx, key=lambda j: clocks[j])
        ggen, pace = regions[i]
        try:
            r0, n = next(ggen)
        except StopIteration:
            idx.remove(i)
            continue
        assert r0 % 4 == 0 and n % 4 == 0
        clocks[i] += pace * n
        for u in range(r0 // 4, (r0 + n) // 4):
            done_at[u] = clocks[i]
        for g in range(H // 8):
            if g in queued:
                continue
            if 2 * g in done_at and 2 * g + 1 in done_at:
                queued.add(g)
                pending.append((max(done_at[2 * g], done_at[2 * g + 1]), g))
        pending.sort()
    assert len(queued) == H // 8, f"unemitted 1x1 groups: {sorted(set(range(16)) - queued)}"


# ---------------------------------------------------------------------------
# host-side entry point
# ---------------------------------------------------------------------------

_PROGRAM_CACHE: dict[str, bass.Bass] = {}


def _get_program() -> bass.Bass:
    if "p" not in _PROGRAM_CACHE:
        _PROGRAM_CACHE["p"] = build_program()
    return _PROGRAM_CACHE["p"]


def _host_prep(inputs: dict):
    import ml_dtypes

    x = np.asarray(inputs["x"], dtype=np.float32)
    d = np.asarray(inputs["d"], dtype=np.float32)
    Wk1 = np.asarray(inputs["Wk1"], dtype=np.float32)
    Wk2 = np.asarray(inputs["Wk2"], dtype=np.float32)
    Wc = np.asarray(inputs["Wc"], dtype=np.float32)
    bc = np.asarray(inputs["bc"], dtype=np.float32)

    wk1t = np.ascontiguousarray(Wk1.T)
    w = Wk2.reshape(C, KK * KK, C).transpose(2, 1, 0)  # (j, t, c)
    wk2td = np.ascontiguousarray(
        np.concatenate([w, w], axis=2).reshape(C, KK * KK * 2 * C)
    )
    wct = np.ascontiguousarray(Wc.T)
    wcbd = np.zeros((2 * C, 2 * C), np.float32)
    wcbd[0:C, 0:C] = wct
    wcbd[C:, C:] = wct
    wcbd = wcbd.astype(ml_dtypes.bfloat16)
    bc2 = np.ascontiguousarray(np.concatenate([bc, bc]).reshape(2 * C, 1))

    xcast = x.astype(ml_dtypes.bfloat16)

    in_maps = []
    for i in range(NCORES):
        xsamp = np.ascontiguousarray(xcast[S * i : S * (i + 1)].reshape(S * C, H * W))
        dT = np.ascontiguousarray(d[S * i : S * (i + 1)].T)
        mlpw = np.concatenate([dT, wk1t, wk2td], axis=1).astype(ml_dtypes.bfloat16)
        in_maps.append(
            {
                "x": xsamp,
                "mlpw": np.ascontiguousarray(mlpw),
                "wcbd": wcbd,
                "bc2": bc2,
            }
        )
    return in_maps


def run_on_hw(inputs: dict, **kwargs):
    from concourse.bass_utils import run_bass_kernel_spmd

    nc = _get_program()
    in_maps = _host_prep(inputs)
    res = run_bass_kernel_spmd(nc, in_maps, core_ids=list(range(NCORES)), **kwargs)
    outs = res.results
    B = S * NCORES
    out = np.empty((B, C, H, W), dtype=np.float32)
    for i in range(NCORES):
        out[S * i : S * (i + 1)] = (
            outs[i]["out"].astype(np.float32).reshape(S, C, H, W)
        )
    return out, res


def kernel(**inputs) -> np.ndarray:
    out, _ = run_on_hw(inputs)
    return out


if __name__ == "__main__":
    nc = build_program()
    print("program built OK")


# revision 8
# speedup vs baseline: 2.8815x; 1.0032x over previous
"""Trainium2 Bass kernel for nn_DA_conv: per-sample dynamic depthwise 3x3 conv
(+LeakyReLU) followed by a 1x1 pointwise conv, with the 3x3 kernels produced by
a small per-sample MLP.

Strategy (8 NeuronCores, batch data-parallel, 2 samples per core):
  - SBUF partition p = (sample s = p//64, channel c = p%64); the feature map
    is resident in bf16 with top/bottom halo rows plus a one-element pad on
    each end (no W padding: edge columns are handled by full-width taps plus
    strided correction matmuls / partial-width vector ops).
  - The image rows are split into three regions, one per compute engine, so
    the depthwise conv runs on PE, DVE and GPSIMD concurrently:
      A  : PE diagonal matmuls (9 full-width PSUM-accumulated taps + 6
           strided W-edge corrections per 4-row bank), ACT Prelu evac.
      B2 : DVE per-tap scale-muls (4x mode) + tensor-tensor adds.
      B3 : gpsimd scalar_tensor_tensor MAC chain.
    LeakyReLU for B2/B3 runs on ACT (Prelu), interleaved by estimated
    readiness so the in-order ACT queue never stalls the PSUM pipeline.
  - 1x1 conv: one 128x128 block-diagonal matmul per PSUM bank covers both
    samples at once; ACT evacuates with the bias via Identity+bias.
  - DMA: input + output both on the otherwise-idle SP queue; compute engines
    never issue DMAs.
"""

import sys

sys.path.insert(0, "/opt/trn_rl_repo")

from contextlib import ExitStack

import numpy as np

import concourse.bacc as bacc
import concourse.bass as bass
import concourse.mybir as mybir
import concourse.tile as tile

S = 2            # samples per core
C = 64           # channels
H = W = 128      # spatial
KK = 3           # conv kernel size
NCORES = 8
XR = H + 2       # resident rows incl halo

f32 = mybir.dt.float32
bf16 = mybir.dt.bfloat16
A = mybir.AluOpType
PRELU = mybir.ActivationFunctionType.Prelu
IDENT = mybir.ActivationFunctionType.Identity

# Row stripes per engine (cover 0..128, multiples of 4).
# Interleaved so 1x1-group completions stagger instead of bunching at the end.
STRIPES_A = [(0, 16), (28, 16), (56, 16), (84, 16), (124, 4)]   # PE, 68 rows
STRIPES_B2 = [(44, 12), (16, 12), (112, 12), (72, 4)]           # DVE, 40 rows
STRIPES_B3 = [(100, 12), (76, 8)]                               # Pool, 20 rows
R_A = sum(n for _, n in STRIPES_A)
R_B2 = sum(n for _, n in STRIPES_B2)
R_B3 = sum(n for _, n in STRIPES_B3)

# virtual-clock pace estimates (ns per image row) for emission ordering
PACE_A = 490.0
PACE_B2 = 890.0
PACE_B3 = 1850.0
K_FILL = 3000.0
K_XACT = [6, 11]     # consume-order chunk indices carried by the ACT queue
K_TAILSPLIT = 4      # last-N 1x1 groups: evac halves split across ACT/DVE
K_ASPLIT = 0
K_LSPLIT = 1    # last B2/B3 chunk: lrelu halves split across ACT/DVE
K_TAILFINE = 0
K_ODMA = 2      # 2nd-to-last group: out DMA via the idle ACT queue

def _lrelu_maybe_split(nc, dst, n, split):
    if split:
        half = (n * W) // 2
        nc.scalar.activation(dst[:, 0:half], dst[:, 0:half],
                             mybir.ActivationFunctionType.Prelu, alpha=0.1)
        nc.vector.scalar_tensor_tensor(
            dst[:, half:], dst[:, half:], 0.1, dst[:, half:],
            op0=A.mult, op1=A.max)
    else:
        nc.scalar.activation(dst, dst, mybir.ActivationFunctionType.Prelu, alpha=0.1)

TAPS = [(di, dj) for di in range(KK) for dj in range(KK)]
CORR = [(di, dj) for dj in (0, 2) for di in range(KK)]


def _tap_geom(dj):
    """(out_col_lo, src_col_lo, ncols) for a horizontal tap shift."""
    if dj == 0:
        return 1, 0, W - 1
    if dj == 1:
        return 0, 0, W
    return 0, 1, W - 1


def build_program() -> bass.Bass:
    nc = bacc.Bacc("TRN2", target_bir_lowering=False, debug=False)

    x_d = nc.dram_tensor("x", [S * C, H * W], bf16, kind="ExternalInput").ap()
    # mlpw = [dT (2) | Wk1.T (64) | wk2td (1152)] in one tensor -> one DMA
    # (wk2td[j, t*128 + s*64 + c] = Wk2[c*9 + t, j], tap-major, sample-dup)
    mlpw_d = nc.dram_tensor(
        "mlpw", [C, S + C + KK * KK * 2 * C], bf16, kind="ExternalInput"
    ).ap()
    wcbd_d = nc.dram_tensor("wcbd", [2 * C, 2 * C], bf16, kind="ExternalInput").ap()
    bc_d = nc.dram_tensor("bc2", [2 * C, 1], f32, kind="ExternalInput").ap()
    out_d = nc.dram_tensor("out", [S * C, H * W], bf16, kind="ExternalOutput").ap()

    with tile.TileContext(nc) as tc, ExitStack() as ctx:
        _body(ctx, tc, x_d, mlpw_d, wcbd_d, bc_d, out_d)
    nc.compile()
    return nc


def _body(ctx, tc, x_d, mlpw_d, wcbd_d, bc_d, out_d):
    nc = tc.nc
    const = ctx.enter_context(tc.tile_pool(name="const", bufs=1))
    xpool = ctx.enter_context(tc.tile_pool(name="xs", bufs=1))
    dwp = ctx.enter_context(tc.tile_pool(name="dw", bufs=1))
    tmpp = ctx.enter_context(tc.tile_pool(name="tmp", bufs=2))
    sbp = ctx.enter_context(tc.tile_pool(name="sb", bufs=2))
    obp = ctx.enter_context(tc.tile_pool(name="ob", bufs=3))
    pdw = ctx.enter_context(tc.tile_pool(name="pdw", bufs=2, space="PSUM"))
    po = ctx.enter_context(tc.tile_pool(name="po", bufs=3, space="PSUM"))

    # ---------------- const loads (qSP, before x) ----------------
    mlpw = const.tile([C, S + C + KK * KK * 2 * C], bf16)
    nc.sync.dma_start(mlpw[:, :], mlpw_d)
    dts = mlpw[:, 0:S]
    wk1t = mlpw[:, S : S + C]
    wk2td = mlpw[:, S + C : S + C + KK * KK * 2 * C]

    # ---------------- resident x (halo rows zeroed, +-1 elem pad) ----------
    xs = xpool.tile([128, XR * W + 2], bf16)
    nc.vector.memset(xs[:, 0 : W + 1], 0.0)
    nc.vector.memset(xs[:, (XR - 1) * W + 1 : XR * W + 2], 0.0)
    XBLK = 8

    def _xchunks():
        """order the 16 aligned 8-row x chunks by estimated consume time."""
        consume_at = {}
        for stripes, pace in ((STRIPES_A, PACE_A), (STRIPES_B2, PACE_B2),
                              (STRIPES_B3, PACE_B3)):
            t = 0.0
            for r0, n in stripes:
                for r in range(r0, r0 + n):
                    consume_at[r] = t + pace * (r - r0)
                t += pace * n

        def consume(k):
            rows = range(max(0, k - 1), min(H, k + XBLK + 1))
            return min(consume_at[r] for r in rows)

        return sorted(range(0, H, XBLK), key=consume)

    wcbd = const.tile([2 * C, 2 * C], bf16)
    bc2 = const.tile([2 * C, 1], f32)
    _xorder = _xchunks()
    # SP carries the early-consumed chunks; ACT takes K_XACT positions after
    # the MLP section (so the in-order ACT queue does the MLP prelu first).
    _xact = [k for ci, k in enumerate(_xorder) if ci in K_XACT]
    _xsp = [k for k in _xorder if k not in _xact]
    for ci, k in enumerate(_xsp):
        nc.sync.dma_start(
            xs[:, (k + 1) * W + 1 : (k + XBLK + 1) * W + 1],
            x_d[:, k * W : (k + XBLK) * W],
        )
        if ci == 3:
            nc.sync.dma_start(wcbd[:, :], wcbd_d)
            nc.sync.dma_start(bc2[:, :], bc_d)

    # dw output (lrelu'd, bf16) == rhs of the 1x1 conv
    dwout = dwp.tile([128, H * W], bf16)



    # ---------------- kernel-generating MLP ----------------
    hid_ps = po.tile([C, S], f32, tag="po")
    nc.tensor.matmul(hid_ps[:, :], lhsT=wk1t, rhs=dts, start=True, stop=True)
    hid_sb = const.tile([C, S], bf16)
    nc.scalar.activation(hid_sb[:, :], hid_ps[:, :], PRELU, alpha=0.1)

    kcols = const.tile([2 * C, KK * KK], f32)
    kps = po.tile([2 * C, KK * KK], f32, tag="po")
    for t in range(KK * KK):
        nc.tensor.matmul(
            kps[0:C, t : t + 1],
            lhsT=wk2td[:, t * 128 : t * 128 + C],
            rhs=hid_sb[:, 0:1],
            start=True, stop=True,
        )
        nc.tensor.matmul(
            kps[C : 2 * C, t : t + 1],
            lhsT=wk2td[:, t * 128 + C : (t + 1) * 128],
            rhs=hid_sb[:, 1:2],
            start=True, stop=True,
        )
    nc.vector.tensor_copy(kcols[:, :], kps[:, :])

    # per-tap diagonal weights for the PE region
    id_i = const.tile([128, 128], mybir.dt.int32)
    nc.gpsimd.iota(id_i[:, :], pattern=[[1, 128]], base=0, channel_multiplier=-1)
    idf = const.tile([128, 128], bf16)
    nc.vector.tensor_scalar(idf[:, :], id_i[:, :], 0, None, A.is_equal)
    diag = const.tile([128, KK * KK * 128], bf16)
    for t in range(KK * KK):
        nc.vector.tensor_scalar_mul(
            diag[:, t * 128 : (t + 1) * 128], idf[:, :], kcols[:, t : t + 1]
        )
    # negated diagonals for the W-edge wrap corrections (taps dj=0 and dj=2)
    negk = const.tile([2 * C, KK * KK], f32)
    nc.vector.tensor_scalar_mul(negk[:, :], kcols[:, :], -1.0)
    negdiag = const.tile([128, 6 * 128], bf16)
    for j, (di, dj) in enumerate(CORR):
        t = di * KK + dj
        nc.vector.tensor_scalar_mul(
            negdiag[:, j * 128 : (j + 1) * 128], idf[:, :], negk[:, t : t + 1]
        )

    for k in _xact:
        nc.scalar.dma_start(
            xs[:, (k + 1) * W + 1 : (k + XBLK + 1) * W + 1],
            x_d[:, k * W : (k + XBLK) * W],
        )

    xs3 = xs[:, 1 : 1 + XR * W].rearrange("p (r w) -> p r w", w=W)  # [128, XR, W]
    dw3 = dwout[:, :].rearrange("p (r w) -> p r w", w=W)

    # ---------------- region emitters ----------------
    def emit_A():
        """PE: 9 full-width taps + 6 strided W-edge corrections per 4-row
        bank; ACT Prelu evacuates to dwout."""
        banks = [r0 + 4 * i for r0, n in STRIPES_A for i in range(n // 4)]
        emit_A.bi = 0
        emit_A.nb = len(banks)
        for rb in banks:
            if True:
                ps = pdw.tile([128, 512], f32, tag="pdw", name=f"pdw{rb}")
                bank = ps[:, :]
                for i, (di, dj) in enumerate(TAPS[:-1]):
                    t = di * KK + dj
                    base = (rb + di) * W + dj
                    nc.tensor.matmul(
                        bank[:, :],
                        lhsT=diag[:, t * 128 : (t + 1) * 128],
                        rhs=xs[:, base : base + 512],
                        start=(i == 0),
                        stop=False,
                    )
                for j, (di, dj) in enumerate(CORR):
                    if dj == 0:
                        dst = bank[:, 0 : 512 : W]
                        c0 = (rb + di) * W
                    else:
                        dst = bank[:, W - 1 : 512 : W]
                        c0 = (rb + di + 1) * W + 1
                    nc.tensor.matmul(
                        dst,
                        lhsT=negdiag[:, j * 128 : (j + 1) * 128],
                        rhs=xs[:, c0 : c0 + 3 * W + 1 : W],
                        start=False,
                        stop=False,
                    )
                di, dj = TAPS[-1]
                t = di * KK + dj
                base = (rb + di) * W + dj
                nc.tensor.matmul(
                    bank[:, :],
                    lhsT=diag[:, t * 128 : (t + 1) * 128],
                    rhs=xs[:, base : base + 512],
                    start=False,
                    stop=True,
                )
            emit_A.bi += 1
            if K_ASPLIT and emit_A.bi > emit_A.nb - K_ASPLIT:
                nc.scalar.activation(
                    dwout[:, rb * W : rb * W + 256], ps[:, 0:256], PRELU, alpha=0.1
                )
                nc.vector.scalar_tensor_tensor(
                    dwout[:, rb * W + 256 : (rb + 4) * W], ps[:, 256:512], 0.1,
                    ps[:, 256:512], op0=A.mult, op1=A.max,
                )
            else:
                nc.scalar.activation(
                    dwout[:, rb * W : (rb + 4) * W], ps[:, :], PRELU, alpha=0.1
                )
            yield rb, 4

    def emit_B2():
        """DVE: scale-mul into tmp (4x), tensor-tensor add into dwout (2x);
        lrelu on ACT."""
        for ci, (r0, n) in enumerate(STRIPES_B2):
            last = ci == len(STRIPES_B2) - 1
            dst = dwout[:, r0 * W : (r0 + n) * W]
            t0 = 0 * KK + 1
            nc.vector.tensor_scalar_mul(
                dst, xs[:, r0 * W + 1 : (r0 + n) * W + 1], kcols[:, t0 : t0 + 1]
            )
            for di, dj in TAPS:
                if (di, dj) == (0, 1):
                    continue
                t = di * KK + dj
                w_lo, s_lo, ncol = _tap_geom(dj)
                tm = tmpp.tile([128, 16 * W], bf16, tag="b2t", name=f"tm{r0}_{t}")
                tm3 = tm[:, :].rearrange("p (r w) -> p r w", w=W)
                src = xs3[:, r0 + di : r0 + di + n, s_lo : s_lo + ncol]
                nc.vector.tensor_scalar_mul(tm3[:, 0:n, 0:ncol], src, kcols[:, t : t + 1])
                nc.vector.tensor_tensor(
                    dw3[:, r0 : r0 + n, w_lo : w_lo + ncol],
                    dw3[:, r0 : r0 + n, w_lo : w_lo + ncol],
                    tm3[:, 0:n, 0:ncol],
                    A.add,
                )
            _lrelu_maybe_split(nc, dst, n,
                               K_LSPLIT and ci >= len(STRIPES_B2) - K_LSPLIT)
            yield r0, n

    def emit_B3():
        """Pool: scale-mul into tmp (TensorScalarPtr) + tensor-tensor add
        into dwout; lrelu on ACT."""
        B3MAX = max(n for _, n in STRIPES_B3)
        for ci, (r0, n) in enumerate(STRIPES_B3):
            dst = dwout[:, r0 * W : (r0 + n) * W]
            t0 = 0 * KK + 1
            nc.gpsimd.tensor_scalar_mul(
                dst, xs[:, r0 * W + 1 : (r0 + n) * W + 1], kcols[:, t0 : t0 + 1]
            )
            for di, dj in TAPS:
                if (di, dj) == (0, 1):
                    continue
                t = di * KK + dj
                w_lo, s_lo, ncol = _tap_geom(dj)
                pt = sbp.tile([128, B3MAX * W], bf16, tag="p3t", name=f"pt{r0}_{t}")
                pt3 = pt[:, :].rearrange("p (r w) -> p r w", w=W)
                src = xs3[:, r0 + di : r0 + di + n, s_lo : s_lo + ncol]
                nc.gpsimd.tensor_scalar_mul(
                    pt3[:, 0:n, 0:ncol], src, kcols[:, t : t + 1]
                )
                nc.gpsimd.tensor_tensor(
                    dw3[:, r0 : r0 + n, w_lo : w_lo + ncol],
                    dw3[:, r0 : r0 + n, w_lo : w_lo + ncol],
                    pt3[:, 0:n, 0:ncol],
                    A.add,
                )
            _lrelu_maybe_split(nc, dst, n,
                               K_LSPLIT and ci >= len(STRIPES_B3) - K_LSPLIT)
            yield r0, n

    # ------- merged emission: dw chunks + 1x1 groups by virtual clocks -----
    def emit_group(g):
        rb = g * 8
        emit_group.count += 1
        if K_TAILFINE and emit_group.count > H // 8 - K_TAILFINE:
            # fine-grained drain: two independent 4-row units, evacs
            # alternating ACT/DVE, separate output DMAs
            for half in range(2):
                off = (rb + 4 * half) * W
                psf = po.tile([128, 512], f32, tag="po", name=f"pof{g}_{half}")
                nc.tensor.matmul(
                    psf[:, :], lhsT=wcbd[:, :], rhs=dwout[:, off : off + 512],
                    start=True, stop=True,
                )
                obf = obp.tile([128, 512], bf16, tag="ob", name=f"obf{g}_{half}")
                if half == 0:
                    nc.scalar.activation(obf[:, :], psf[:, :], IDENT, bias=bc2[:, 0:1])
                else:
                    nc.vector.tensor_scalar_add(obf[:, :], psf[:, :], bc2[:, 0:1])
                nc.sync.dma_start(out_d[:, off : off + 512], obf[:, :])
            return
        ps1 = po.tile([128, 1024], f32, tag="po", name=f"po{g}")
        for half in range(2):
            off = (rb + 4 * half) * W
            nc.tensor.matmul(
                ps1[:, half * 512 : (half + 1) * 512],
                lhsT=wcbd[:, :],
                rhs=dwout[:, off : off + 512],
                start=True, stop=True,
            )
        ob = obp.tile([128, 1024], bf16, tag="ob", name=f"ob{g}")
        if K_TAILSPLIT and emit_group.count > H // 8 - K_TAILSPLIT:
            # split the evac: ACT and DVE each do one 512 half concurrently
            nc.scalar.activation(ob[:, 0:512], ps1[:, 0:512], IDENT, bias=bc2[:, 0:1])
            nc.vector.tensor_scalar_add(ob[:, 512:1024], ps1[:, 512:1024], bc2[:, 0:1])
        else:
            nc.scalar.activation(ob[:, :], ps1[:, :], IDENT, bias=bc2[:, 0:1])
        if K_ODMA and emit_group.count > H // 8 - K_ODMA and emit_group.count % 2 == 0:
            nc.scalar.dma_start(out_d[:, rb * W : (rb + 8) * W], ob[:, :])
        else:
            nc.sync.dma_start(out_d[:, rb * W : (rb + 8) * W], ob[:, :])

    emit_group.count = 0
    FILL = K_FILL  # rough ns before first dw work can start
    regions = [
        (emit_A(), PACE_A),
        (emit_B2(), PACE_B2),
        (emit_B3(), PACE_B3),
    ]
    clocks = [FILL] * len(regions)
    done_at: dict[int, float] = {}  # 4-row unit index -> est completion
    pending: list[tuple[float, int]] = []  # (ready est, 1x1 group)
    queued: set[int] = set()
    idx = list(range(len(regions)))
    while idx or pending:
        tnext = min((clocks[j] for j in idx), default=1e18)
        if pending and pending[0][0] <= tnext:
            _, g = pending.pop(0)
            emit_group(g)
            continue
        if not idx:
            continue
        i = min(idx, key=lambda j: clocks[j])
        ggen, pace = regions[i]
        try:
            r0, n = next(ggen)
        except StopIteration:
            idx.remove(i)
            continue
        assert r0 % 4 == 0 and n % 4 == 0
        clocks[i] += pace * n
        for u in range(r0 // 4, (r0 + n) // 4):
            done_at[u] = clocks[i]
        for g in range(H // 8):
            if g in queued:
                continue
            if 2 * g in done_at and 2 * g + 1 in done_at:
                queued.add(g)
                pending.append((max(done_at[2 * g], done_at[2 * g + 1]), g))
        pending.sort()
    assert len(queued) == H // 8, f"unemitted 1x1 groups: {sorted(set(range(16)) - queued)}"


# ---------------------------------------------------------------------------
# host-side entry point
# ---------------------------------------------------------------------------

_PROGRAM_CACHE: dict[str, bass.Bass] = {}


def _get_program() -> bass.Bass:
    if "p" not in _PROGRAM_CACHE:
        _PROGRAM_CACHE["p"] = build_program()
    return _PROGRAM_CACHE["p"]


def _host_prep(inputs: dict):
    import ml_dtypes

    x = np.asarray(inputs["x"], dtype=np.float32)
    d = np.asarray(inputs["d"], dtype=np.float32)
    Wk1 = np.asarray(inputs["Wk1"], dtype=np.float32)
    Wk2 = np.asarray(inputs["Wk2"], dtype=np.float32)
    Wc = np.asarray(inputs["Wc"], dtype=np.float32)
    bc = np.asarray(inputs["bc"], dtype=np.float32)

    wk1t = np.ascontiguousarray(Wk1.T)
    w = Wk2.reshape(C, KK * KK, C).transpose(2, 1, 0)  # (j, t, c)
    wk2td = np.ascontiguousarray(
        np.concatenate([w, w], axis=2).reshape(C, KK * KK * 2 * C)
    )
    wct = np.ascontiguousarray(Wc.T)
    wcbd = np.zeros((2 * C, 2 * C), np.float32)
    wcbd[0:C, 0:C] = wct
    wcbd[C:, C:] = wct
    wcbd = wcbd.astype(ml_dtypes.bfloat16)
    bc2 = np.ascontiguousarray(np.concatenate([bc, bc]).reshape(2 * C, 1))

    xcast = x.astype(ml_dtypes.bfloat16)

    in_maps = []
    for i in range(NCORES):
        xsamp = np.ascontiguousarray(xcast[S * i : S * (i + 1)].reshape(S * C, H * W))
        dT = np.ascontiguousarray(d[S * i : S * (i + 1)].T)
        mlpw = np.concatenate([dT, wk1t, wk2td], axis=1).astype(ml_dtypes.bfloat16)
        in_maps.append(
            {
                "x": xsamp,
                "mlpw": np.ascontiguousarray(mlpw),
                "wcbd": wcbd,
                "bc2": bc2,
            }
        )
    return in_maps


def run_on_hw(inputs: dict, **kwargs):
    from concourse.bass_utils import run_bass_kernel_spmd

    nc = _get_program()
    in_maps = _host_prep(inputs)
    res = run_bass_kernel_spmd(nc, in_maps, core_ids=list(range(NCORES)), **kwargs)
    outs = res.results
    B = S * NCORES
    out = np.empty((B, C, H, W), dtype=np.float32)
    for i in range(NCORES):
        out[S * i : S * (i + 1)] = (
            outs[i]["out"].astype(np.float32).reshape(S, C, H, W)
        )
    return out, res


def kernel(**inputs) -> np.ndarray:
    out, _ = run_on_hw(inputs)
    return out


if __name__ == "__main__":
    nc = build_program()
    print("program built OK")


# revision 9
# speedup vs baseline: 2.8970x; 1.0054x over previous
"""Trainium2 Bass kernel for nn_DA_conv: per-sample dynamic depthwise 3x3 conv
(+LeakyReLU) followed by a 1x1 pointwise conv, with the 3x3 kernels produced by
a small per-sample MLP.

Strategy (8 NeuronCores, batch data-parallel, 2 samples per core):
  - SBUF partition p = (sample s = p//64, channel c = p%64); the feature map
    is resident in bf16 with top/bottom halo rows plus a one-element pad on
    each end (no W padding: edge columns are handled by full-width taps plus
    strided correction matmuls / partial-width vector ops).
  - The image rows are split into three regions, one per compute engine, so
    the depthwise conv runs on PE, DVE and GPSIMD concurrently:
      A  : PE diagonal matmuls (9 full-width PSUM-accumulated taps + 6
           strided W-edge corrections per 4-row bank), ACT Prelu evac.
      B2 : DVE per-tap scale-muls (4x mode) + tensor-tensor adds.
      B3 : gpsimd scalar_tensor_tensor MAC chain.
    LeakyReLU for B2/B3 runs on ACT (Prelu), interleaved by estimated
    readiness so the in-order ACT queue never stalls the PSUM pipeline.
  - 1x1 conv: one 128x128 block-diagonal matmul per PSUM bank covers both
    samples at once; ACT evacuates with the bias via Identity+bias.
  - DMA: input + output both on the otherwise-idle SP queue; compute engines
    never issue DMAs.
"""

import sys

sys.path.insert(0, "/opt/trn_rl_repo")

from contextlib import ExitStack

import numpy as np

import concourse.bacc as bacc
import concourse.bass as bass
import concourse.mybir as mybir
import concourse.tile as tile

S = 2            # samples per core
C = 64           # channels
H = W = 128      # spatial
KK = 3           # conv kernel size
NCORES = 8
XR = H + 2       # resident rows incl halo

f32 = mybir.dt.float32
bf16 = mybir.dt.bfloat16
A = mybir.AluOpType
PRELU = mybir.ActivationFunctionType.Prelu
IDENT = mybir.ActivationFunctionType.Identity

# Row stripes per engine (cover 0..128, multiples of 4).
# Interleaved so 1x1-group completions stagger instead of bunching at the end.
STRIPES_A = [(0, 16), (28, 16), (56, 16), (84, 16), (124, 4)]   # PE, 68 rows
STRIPES_B2 = [(44, 12), (16, 12), (112, 12), (72, 4)]           # DVE, 40 rows
STRIPES_B3 = [(100, 12), (76, 8)]                               # Pool, 20 rows
R_A = sum(n for _, n in STRIPES_A)
R_B2 = sum(n for _, n in STRIPES_B2)
R_B3 = sum(n for _, n in STRIPES_B3)

# virtual-clock pace estimates (ns per image row) for emission ordering
PACE_A = 490.0
PACE_B2 = 890.0
PACE_B3 = 1850.0
K_FILL = 3000.0
K_XACT = [6, 11]     # consume-order chunk indices carried by the ACT queue
K_TAILSPLIT = 4      # last-N 1x1 groups: evac halves split across ACT/DVE
K_ASPLIT = 0
K_LSPLIT = 1    # last B2/B3 chunk: lrelu halves split across ACT/DVE
K_TAILFINE = 0
K_ODMA = 2      # last group: out DMA via the idle ACT queue
K_ODMAPAR = 0
K_SEED = 5  # emission-order jitter seed (best of 21-seed search)

def _lrelu_maybe_split(nc, dst, n, split):
    if split:
        half = (n * W) // 2
        nc.scalar.activation(dst[:, 0:half], dst[:, 0:half],
                             mybir.ActivationFunctionType.Prelu, alpha=0.1)
        nc.vector.scalar_tensor_tensor(
            dst[:, half:], dst[:, half:], 0.1, dst[:, half:],
            op0=A.mult, op1=A.max)
    else:
        nc.scalar.activation(dst, dst, mybir.ActivationFunctionType.Prelu, alpha=0.1)

TAPS = [(di, dj) for di in range(KK) for dj in range(KK)]
CORR = [(di, dj) for dj in (0, 2) for di in range(KK)]


def _tap_geom(dj):
    """(out_col_lo, src_col_lo, ncols) for a horizontal tap shift."""
    if dj == 0:
        return 1, 0, W - 1
    if dj == 1:
        return 0, 0, W
    return 0, 1, W - 1


def build_program() -> bass.Bass:
    nc = bacc.Bacc("TRN2", target_bir_lowering=False, debug=False)

    x_d = nc.dram_tensor("x", [S * C, H * W], bf16, kind="ExternalInput").ap()
    # mlpw = [dT (2) | Wk1.T (64) | wk2td (1152)] in one tensor -> one DMA
    # (wk2td[j, t*128 + s*64 + c] = Wk2[c*9 + t, j], tap-major, sample-dup)
    mlpw_d = nc.dram_tensor(
        "mlpw", [C, S + C + KK * KK * 2 * C], bf16, kind="ExternalInput"
    ).ap()
    wcbd_d = nc.dram_tensor("wcbd", [2 * C, 2 * C], bf16, kind="ExternalInput").ap()
    bc_d = nc.dram_tensor("bc2", [2 * C, 1], f32, kind="ExternalInput").ap()
    out_d = nc.dram_tensor("out", [S * C, H * W], bf16, kind="ExternalOutput").ap()

    with tile.TileContext(nc) as tc, ExitStack() as ctx:
        _body(ctx, tc, x_d, mlpw_d, wcbd_d, bc_d, out_d)
    nc.compile()
    return nc


def _body(ctx, tc, x_d, mlpw_d, wcbd_d, bc_d, out_d):
    nc = tc.nc
    const = ctx.enter_context(tc.tile_pool(name="const", bufs=1))
    xpool = ctx.enter_context(tc.tile_pool(name="xs", bufs=1))
    dwp = ctx.enter_context(tc.tile_pool(name="dw", bufs=1))
    tmpp = ctx.enter_context(tc.tile_pool(name="tmp", bufs=2))
    sbp = ctx.enter_context(tc.tile_pool(name="sb", bufs=2))
    obp = ctx.enter_context(tc.tile_pool(name="ob", bufs=3))
    pdw = ctx.enter_context(tc.tile_pool(name="pdw", bufs=2, space="PSUM"))
    po = ctx.enter_context(tc.tile_pool(name="po", bufs=3, space="PSUM"))

    # ---------------- const loads (qSP, before x) ----------------
    mlpw = const.tile([C, S + C + KK * KK * 2 * C], bf16)
    nc.sync.dma_start(mlpw[:, :], mlpw_d)
    dts = mlpw[:, 0:S]
    wk1t = mlpw[:, S : S + C]
    wk2td = mlpw[:, S + C : S + C + KK * KK * 2 * C]

    # ---------------- resident x (halo rows zeroed, +-1 elem pad) ----------
    xs = xpool.tile([128, XR * W + 2], bf16)
    nc.vector.memset(xs[:, 0 : W + 1], 0.0)
    nc.vector.memset(xs[:, (XR - 1) * W + 1 : XR * W + 2], 0.0)
    XBLK = 8

    def _xchunks():
        """order the 16 aligned 8-row x chunks by estimated consume time."""
        consume_at = {}
        for stripes, pace in ((STRIPES_A, PACE_A), (STRIPES_B2, PACE_B2),
                              (STRIPES_B3, PACE_B3)):
            t = 0.0
            for r0, n in stripes:
                for r in range(r0, r0 + n):
                    consume_at[r] = t + pace * (r - r0)
                t += pace * n

        def consume(k):
            rows = range(max(0, k - 1), min(H, k + XBLK + 1))
            return min(consume_at[r] for r in rows)

        return sorted(range(0, H, XBLK), key=consume)

    wcbd = const.tile([2 * C, 2 * C], bf16)
    bc2 = const.tile([2 * C, 1], f32)
    _xorder = _xchunks()
    # SP carries the early-consumed chunks; ACT takes K_XACT positions after
    # the MLP section (so the in-order ACT queue does the MLP prelu first).
    _xact = [k for ci, k in enumerate(_xorder) if ci in K_XACT]
    _xsp = [k for k in _xorder if k not in _xact]
    for ci, k in enumerate(_xsp):
        nc.sync.dma_start(
            xs[:, (k + 1) * W + 1 : (k + XBLK + 1) * W + 1],
            x_d[:, k * W : (k + XBLK) * W],
        )
        if ci == 3:
            nc.sync.dma_start(wcbd[:, :], wcbd_d)
            nc.sync.dma_start(bc2[:, :], bc_d)

    # dw output (lrelu'd, bf16) == rhs of the 1x1 conv
    dwout = dwp.tile([128, H * W], bf16)



    # ---------------- kernel-generating MLP ----------------
    hid_ps = po.tile([C, S], f32, tag="po")
    nc.tensor.matmul(hid_ps[:, :], lhsT=wk1t, rhs=dts, start=True, stop=True)
    hid_sb = const.tile([C, S], bf16)
    nc.scalar.activation(hid_sb[:, :], hid_ps[:, :], PRELU, alpha=0.1)

    kcols = const.tile([2 * C, KK * KK], f32)
    kps = po.tile([2 * C, KK * KK], f32, tag="po")
    for t in range(KK * KK):
        nc.tensor.matmul(
            kps[0:C, t : t + 1],
            lhsT=wk2td[:, t * 128 : t * 128 + C],
            rhs=hid_sb[:, 0:1],
            start=True, stop=True,
        )
        nc.tensor.matmul(
            kps[C : 2 * C, t : t + 1],
            lhsT=wk2td[:, t * 128 + C : (t + 1) * 128],
            rhs=hid_sb[:, 1:2],
            start=True, stop=True,
        )
    nc.vector.tensor_copy(kcols[:, :], kps[:, :])

    # per-tap diagonal weights for the PE region
    id_i = const.tile([128, 128], mybir.dt.int32)
    nc.gpsimd.iota(id_i[:, :], pattern=[[1, 128]], base=0, channel_multiplier=-1)
    idf = const.tile([128, 128], bf16)
    nc.vector.tensor_scalar(idf[:, :], id_i[:, :], 0, None, A.is_equal)
    diag = const.tile([128, KK * KK * 128], bf16)
    for t in range(KK * KK):
        nc.vector.tensor_scalar_mul(
            diag[:, t * 128 : (t + 1) * 128], idf[:, :], kcols[:, t : t + 1]
        )
    # negated diagonals for the W-edge wrap corrections (taps dj=0 and dj=2)
    negk = const.tile([2 * C, KK * KK], f32)
    nc.vector.tensor_scalar_mul(negk[:, :], kcols[:, :], -1.0)
    negdiag = const.tile([128, 6 * 128], bf16)
    for j, (di, dj) in enumerate(CORR):
        t = di * KK + dj
        nc.vector.tensor_scalar_mul(
            negdiag[:, j * 128 : (j + 1) * 128], idf[:, :], negk[:, t : t + 1]
        )

    for k in _xact:
        nc.scalar.dma_start(
            xs[:, (k + 1) * W + 1 : (k + XBLK + 1) * W + 1],
            x_d[:, k * W : (k + XBLK) * W],
        )

    xs3 = xs[:, 1 : 1 + XR * W].rearrange("p (r w) -> p r w", w=W)  # [128, XR, W]
    dw3 = dwout[:, :].rearrange("p (r w) -> p r w", w=W)

    # ---------------- region emitters ----------------
    def emit_A():
        """PE: 9 full-width taps + 6 strided W-edge corrections per 4-row
        bank; ACT Prelu evacuates to dwout."""
        banks = [r0 + 4 * i for r0, n in STRIPES_A for i in range(n // 4)]
        emit_A.bi = 0
        emit_A.nb = len(banks)
        for rb in banks:
            if True:
                ps = pdw.tile([128, 512], f32, tag="pdw", name=f"pdw{rb}")
                bank = ps[:, :]
                for i, (di, dj) in enumerate(TAPS[:-1]):
                    t = di * KK + dj
                    base = (rb + di) * W + dj
                    nc.tensor.matmul(
                        bank[:, :],
                        lhsT=diag[:, t * 128 : (t + 1) * 128],
                        rhs=xs[:, base : base + 512],
                        start=(i == 0),
                        stop=False,
                    )
                for j, (di, dj) in enumerate(CORR):
                    if dj == 0:
                        dst = bank[:, 0 : 512 : W]
                        c0 = (rb + di) * W
                    else:
                        dst = bank[:, W - 1 : 512 : W]
                        c0 = (rb + di + 1) * W + 1
                    nc.tensor.matmul(
                        dst,
                        lhsT=negdiag[:, j * 128 : (j + 1) * 128],
                        rhs=xs[:, c0 : c0 + 3 * W + 1 : W],
                        start=False,
                        stop=False,
                    )
                di, dj = TAPS[-1]
                t = di * KK + dj
                base = (rb + di) * W + dj
                nc.tensor.matmul(
                    bank[:, :],
                    lhsT=diag[:, t * 128 : (t + 1) * 128],
                    rhs=xs[:, base : base + 512],
                    start=False,
                    stop=True,
                )
            emit_A.bi += 1
            if K_ASPLIT and emit_A.bi > emit_A.nb - K_ASPLIT:
                nc.scalar.activation(
                    dwout[:, rb * W : rb * W + 256], ps[:, 0:256], PRELU, alpha=0.1
                )
                nc.vector.scalar_tensor_tensor(
                    dwout[:, rb * W + 256 : (rb + 4) * W], ps[:, 256:512], 0.1,
                    ps[:, 256:512], op0=A.mult, op1=A.max,
                )
            else:
                nc.scalar.activation(
                    dwout[:, rb * W : (rb + 4) * W], ps[:, :], PRELU, alpha=0.1
                )
            yield rb, 4

    def emit_B2():
        """DVE: scale-mul into tmp (4x), tensor-tensor add into dwout (2x);
        lrelu on ACT."""
        for ci, (r0, n) in enumerate(STRIPES_B2):
            last = ci == len(STRIPES_B2) - 1
            dst = dwout[:, r0 * W : (r0 + n) * W]
            t0 = 0 * KK + 1
            nc.vector.tensor_scalar_mul(
                dst, xs[:, r0 * W + 1 : (r0 + n) * W + 1], kcols[:, t0 : t0 + 1]
            )
            for di, dj in TAPS:
                if (di, dj) == (0, 1):
                    continue
                t = di * KK + dj
                w_lo, s_lo, ncol = _tap_geom(dj)
                tm = tmpp.tile([128, 16 * W], bf16, tag="b2t", name=f"tm{r0}_{t}")
                tm3 = tm[:, :].rearrange("p (r w) -> p r w", w=W)
                src = xs3[:, r0 + di : r0 + di + n, s_lo : s_lo + ncol]
                nc.vector.tensor_scalar_mul(tm3[:, 0:n, 0:ncol], src, kcols[:, t : t + 1])
                nc.vector.tensor_tensor(
                    dw3[:, r0 : r0 + n, w_lo : w_lo + ncol],
                    dw3[:, r0 : r0 + n, w_lo : w_lo + ncol],
                    tm3[:, 0:n, 0:ncol],
                    A.add,
                )
            _lrelu_maybe_split(nc, dst, n,
                               K_LSPLIT and ci >= len(STRIPES_B2) - K_LSPLIT)
            yield r0, n

    def emit_B3():
        """Pool: scale-mul into tmp (TensorScalarPtr) + tensor-tensor add
        into dwout; lrelu on ACT."""
        B3MAX = max(n for _, n in STRIPES_B3)
        for ci, (r0, n) in enumerate(STRIPES_B3):
            dst = dwout[:, r0 * W : (r0 + n) * W]
            t0 = 0 * KK + 1
            nc.gpsimd.tensor_scalar_mul(
                dst, xs[:, r0 * W + 1 : (r0 + n) * W + 1], kcols[:, t0 : t0 + 1]
            )
            for di, dj in TAPS:
                if (di, dj) == (0, 1):
                    continue
                t = di * KK + dj
                w_lo, s_lo, ncol = _tap_geom(dj)
                pt = sbp.tile([128, B3MAX * W], bf16, tag="p3t", name=f"pt{r0}_{t}")
                pt3 = pt[:, :].rearrange("p (r w) -> p r w", w=W)
                src = xs3[:, r0 + di : r0 + di + n, s_lo : s_lo + ncol]
                nc.gpsimd.tensor_scalar_mul(
                    pt3[:, 0:n, 0:ncol], src, kcols[:, t : t + 1]
                )
                nc.gpsimd.tensor_tensor(
                    dw3[:, r0 : r0 + n, w_lo : w_lo + ncol],
                    dw3[:, r0 : r0 + n, w_lo : w_lo + ncol],
                    pt3[:, 0:n, 0:ncol],
                    A.add,
                )
            _lrelu_maybe_split(nc, dst, n,
                               K_LSPLIT and ci >= len(STRIPES_B3) - K_LSPLIT)
            yield r0, n

    # ------- merged emission: dw chunks + 1x1 groups by virtual clocks -----
    def emit_group(g):
        rb = g * 8
        emit_group.count += 1
        if K_TAILFINE and emit_group.count > H // 8 - K_TAILFINE:
            # fine-grained drain: two independent 4-row units, evacs
            # alternating ACT/DVE, separate output DMAs
            for half in range(2):
                off = (rb + 4 * half) * W
                psf = po.tile([128, 512], f32, tag="po", name=f"pof{g}_{half}")
                nc.tensor.matmul(
                    psf[:, :], lhsT=wcbd[:, :], rhs=dwout[:, off : off + 512],
                    start=True, stop=True,
                )
                obf = obp.tile([128, 512], bf16, tag="ob", name=f"obf{g}_{half}")
                if half == 0:
                    nc.scalar.activation(obf[:, :], psf[:, :], IDENT, bias=bc2[:, 0:1])
                else:
                    nc.vector.tensor_scalar_add(obf[:, :], psf[:, :], bc2[:, 0:1])
                nc.sync.dma_start(out_d[:, off : off + 512], obf[:, :])
            return
        ps1 = po.tile([128, 1024], f32, tag="po", name=f"po{g}")
        for half in range(2):
            off = (rb + 4 * half) * W
            nc.tensor.matmul(
                ps1[:, half * 512 : (half + 1) * 512],
                lhsT=wcbd[:, :],
                rhs=dwout[:, off : off + 512],
                start=True, stop=True,
            )
        ob = obp.tile([128, 1024], bf16, tag="ob", name=f"ob{g}")
        if K_TAILSPLIT and emit_group.count > H // 8 - K_TAILSPLIT:
            # split the evac: ACT and DVE each do one 512 half concurrently
            nc.scalar.activation(ob[:, 0:512], ps1[:, 0:512], IDENT, bias=bc2[:, 0:1])
            nc.vector.tensor_scalar_add(ob[:, 512:1024], ps1[:, 512:1024], bc2[:, 0:1])
        else:
            nc.scalar.activation(ob[:, :], ps1[:, :], IDENT, bias=bc2[:, 0:1])
        if K_ODMA and emit_group.count > H // 8 - K_ODMA and emit_group.count % 2 == K_ODMAPAR:
            nc.scalar.dma_start(out_d[:, rb * W : (rb + 8) * W], ob[:, :])
        else:
            nc.sync.dma_start(out_d[:, rb * W : (rb + 8) * W], ob[:, :])

    emit_group.count = 0
    FILL = K_FILL  # rough ns before first dw work can start
    import random as _random
    _rng = _random.Random(K_SEED) if K_SEED else None
    regions = [
        (emit_A(), PACE_A),
        (emit_B2(), PACE_B2),
        (emit_B3(), PACE_B3),
    ]
    clocks = [FILL] * len(regions)
    done_at: dict[int, float] = {}  # 4-row unit index -> est completion
    pending: list[tuple[float, int]] = []  # (ready est, 1x1 group)
    queued: set[int] = set()
    idx = list(range(len(regions)))
    while idx or pending:
        tnext = min((clocks[j] for j in idx), default=1e18)
        if pending and pending[0][0] <= tnext:
            _, g = pending.pop(0)
            emit_group(g)
            continue
        if not idx:
            continue
        i = min(idx, key=lambda j: clocks[j])
        ggen, pace = regions[i]
        try:
            r0, n = next(ggen)
        except StopIteration:
            idx.remove(i)
            continue
        assert r0 % 4 == 0 and n % 4 == 0
        jit = 1.0 + (_rng.uniform(-0.18, 0.18) if _rng else 0.0)
        clocks[i] += pace * n * jit
        for u in range(r0 // 4, (r0 + n) // 4):
            done_at[u] = clocks[i]
        for g in range(H // 8):
            if g in queued:
                continue
            if 2 * g in done_at and 2 * g + 1 in done_at:
                queued.add(g)
                pending.append((max(done_at[2 * g], done_at[2 * g + 1]), g))
        pending.sort()
    assert len(queued) == H // 8, f"unemitted 1x1 groups: {sorted(set(range(16)) - queued)}"


# ---------------------------------------------------------------------------
# host-side entry point
# ---------------------------------------------------------------------------

_PROGRAM_CACHE: dict[str, bass.Bass] = {}


def _get_program() -> bass.Bass:
    if "p" not in _PROGRAM_CACHE:
        _PROGRAM_CACHE["p"] = build_program()
    return _PROGRAM_CACHE["p"]


def _host_prep(inputs: dict):
    import ml_dtypes

    x = np.asarray(inputs["x"], dtype=np.float32)
    d = np.asarray(inputs["d"], dtype=np.float32)
    Wk1 = np.asarray(inputs["Wk1"], dtype=np.float32)
    Wk2 = np.asarray(inputs["Wk2"], dtype=np.float32)
    Wc = np.asarray(inputs["Wc"], dtype=np.float32)
    bc = np.asarray(inputs["bc"], dtype=np.float32)

    wk1t = np.ascontiguousarray(Wk1.T)
    w = Wk2.reshape(C, KK * KK, C).transpose(2, 1, 0)  # (j, t, c)
    wk2td = np.ascontiguousarray(
        np.concatenate([w, w], axis=2).reshape(C, KK * KK * 2 * C)
    )
    wct = np.ascontiguousarray(Wc.T)
    wcbd = np.zeros((2 * C, 2 * C), np.float32)
    wcbd[0:C, 0:C] = wct
    wcbd[C:, C:] = wct
    wcbd = wcbd.astype(ml_dtypes.bfloat16)
    bc2 = np.ascontiguousarray(np.concatenate([bc, bc]).reshape(2 * C, 1))

    xcast = x.astype(ml_dtypes.bfloat16)

    in_maps = []
    for i in range(NCORES):
        xsamp = np.ascontiguousarray(xcast[S * i : S * (i + 1)].reshape(S * C, H * W))
        dT = np.ascontiguousarray(d[S * i : S * (i + 1)].T)
        mlpw = np.concatenate([dT, wk1t, wk2td], axis=1).astype(ml_dtypes.bfloat16)
        in_maps.append(
            {
                "x": xsamp,
                "mlpw": np.ascontiguousarray(mlpw),
                "wcbd": wcbd,
                "bc2": bc2,
            }
        )
    return in_maps


def run_on_hw(inputs: dict, **kwargs):
    from concourse.bass_utils import run_bass_kernel_spmd

    nc = _get_program()
    in_maps = _host_prep(inputs)
    res = run_bass_kernel_spmd(nc, in_maps, core_ids=list(range(NCORES)), **kwargs)
    outs = res.results
    B = S * NCORES
    out = np.empty((B, C, H, W), dtype=np.float32)
    for i in range(NCORES):
        out[S * i : S * (i + 1)] = (
            outs[i]["out"].astype(np.float32).reshape(S, C, H, W)
        )
    return out, res


def kernel(**inputs) -> np.ndarray:
    out, _ = run_on_hw(inputs)
    return out


if __name__ == "__main__":
    nc = build_program()
    print("program built OK")


# revision 10
# speedup vs baseline: 2.9002x; 1.0011x over previous
"""Trainium2 Bass kernel for nn_DA_conv: per-sample dynamic depthwise 3x3 conv
(+LeakyReLU) followed by a 1x1 pointwise conv, with the 3x3 kernels produced by
a small per-sample MLP.

Strategy (8 NeuronCores, batch data-parallel, 2 samples per core):
  - SBUF partition p = (sample s = p//64, channel c = p%64); the feature map
    is resident in bf16 with top/bottom halo rows plus a one-element pad on
    each end (no W padding: edge columns are handled by full-width taps plus
    strided correction matmuls / partial-width vector ops).
  - The image rows are split into three regions, one per compute engine, so
    the depthwise conv runs on PE, DVE and GPSIMD concurrently:
      A  : PE diagonal matmuls (9 full-width PSUM-accumulated taps + 6
           strided W-edge corrections per 4-row bank), ACT Prelu evac.
      B2 : DVE per-tap scale-muls (4x mode) + tensor-tensor adds.
      B3 : gpsimd scalar_tensor_tensor MAC chain.
    LeakyReLU for B2/B3 runs on ACT (Prelu), interleaved by estimated
    readiness so the in-order ACT queue never stalls the PSUM pipeline.
  - 1x1 conv: one 128x128 block-diagonal matmul per PSUM bank covers both
    samples at once; ACT evacuates with the bias via Identity+bias.
  - DMA: input + output both on the otherwise-idle SP queue; compute engines
    never issue DMAs.
"""

import sys

sys.path.insert(0, "/opt/trn_rl_repo")

from contextlib import ExitStack

import numpy as np

import concourse.bacc as bacc
import concourse.bass as bass
import concourse.mybir as mybir
import concourse.tile as tile

S = 2            # samples per core
C = 64           # channels
H = W = 128      # spatial
KK = 3           # conv kernel size
NCORES = 8
XR = H + 2       # resident rows incl halo

f32 = mybir.dt.float32
bf16 = mybir.dt.bfloat16
A = mybir.AluOpType
PRELU = mybir.ActivationFunctionType.Prelu
IDENT = mybir.ActivationFunctionType.Identity

# Row stripes per engine (cover 0..128, multiples of 4).
# Interleaved so 1x1-group completions stagger instead of bunching at the end.
STRIPES_A = [(0, 16), (28, 16), (56, 16), (84, 16), (124, 4)]   # PE, 68 rows
STRIPES_B2 = [(44, 12), (16, 12), (112, 12), (72, 4)]           # DVE, 40 rows
STRIPES_B3 = [(100, 12), (76, 8)]                               # Pool, 20 rows
R_A = sum(n for _, n in STRIPES_A)
R_B2 = sum(n for _, n in STRIPES_B2)
R_B3 = sum(n for _, n in STRIPES_B3)

# virtual-clock pace estimates (ns per image row) for emission ordering
PACE_A = 490.0
PACE_B2 = 890.0
PACE_B3 = 1850.0
K_FILL = 3000.0
K_XACT = [6, 11]     # consume-order chunk indices carried by the ACT queue
K_TAILSPLIT = 4      # last-N 1x1 groups: evac halves split across ACT/DVE
K_ASPLIT = 0
K_LSPLIT = 1    # last B2/B3 chunk: lrelu halves split across ACT/DVE
K_TAILFINE = 0
K_ODMA = 2      # last group: out DMA via the idle ACT queue
K_ODMAPAR = 0
K_SEED = 5    # emission-order jitter seed (best of 40+ seed/amplitude search)
K_AMP = 0.40

def _lrelu_maybe_split(nc, dst, n, split):
    if split:
        half = (n * W) // 2
        nc.scalar.activation(dst[:, 0:half], dst[:, 0:half],
                             mybir.ActivationFunctionType.Prelu, alpha=0.1)
        nc.vector.scalar_tensor_tensor(
            dst[:, half:], dst[:, half:], 0.1, dst[:, half:],
            op0=A.mult, op1=A.max)
    else:
        nc.scalar.activation(dst, dst, mybir.ActivationFunctionType.Prelu, alpha=0.1)

TAPS = [(di, dj) for di in range(KK) for dj in range(KK)]
CORR = [(di, dj) for dj in (0, 2) for di in range(KK)]


def _tap_geom(dj):
    """(out_col_lo, src_col_lo, ncols) for a horizontal tap shift."""
    if dj == 0:
        return 1, 0, W - 1
    if dj == 1:
        return 0, 0, W
    return 0, 1, W - 1


def build_program() -> bass.Bass:
    nc = bacc.Bacc("TRN2", target_bir_lowering=False, debug=False)

    x_d = nc.dram_tensor("x", [S * C, H * W], bf16, kind="ExternalInput").ap()
    # mlpw = [dT (2) | Wk1.T (64) | wk2td (1152)] in one tensor -> one DMA
    # (wk2td[j, t*128 + s*64 + c] = Wk2[c*9 + t, j], tap-major, sample-dup)
    mlpw_d = nc.dram_tensor(
        "mlpw", [C, S + C + KK * KK * 2 * C], bf16, kind="ExternalInput"
    ).ap()
    wcbd_d = nc.dram_tensor("wcbd", [2 * C, 2 * C], bf16, kind="ExternalInput").ap()
    bc_d = nc.dram_tensor("bc2", [2 * C, 1], f32, kind="ExternalInput").ap()
    out_d = nc.dram_tensor("out", [S * C, H * W], bf16, kind="ExternalOutput").ap()

    with tile.TileContext(nc) as tc, ExitStack() as ctx:
        _body(ctx, tc, x_d, mlpw_d, wcbd_d, bc_d, out_d)
    nc.compile()
    return nc


def _body(ctx, tc, x_d, mlpw_d, wcbd_d, bc_d, out_d):
    nc = tc.nc
    const = ctx.enter_context(tc.tile_pool(name="const", bufs=1))
    xpool = ctx.enter_context(tc.tile_pool(name="xs", bufs=1))
    dwp = ctx.enter_context(tc.tile_pool(name="dw", bufs=1))
    tmpp = ctx.enter_context(tc.tile_pool(name="tmp", bufs=2))
    sbp = ctx.enter_context(tc.tile_pool(name="sb", bufs=2))
    obp = ctx.enter_context(tc.tile_pool(name="ob", bufs=3))
    pdw = ctx.enter_context(tc.tile_pool(name="pdw", bufs=2, space="PSUM"))
    po = ctx.enter_context(tc.tile_pool(name="po", bufs=3, space="PSUM"))

    # ---------------- const loads (qSP, before x) ----------------
    mlpw = const.tile([C, S + C + KK * KK * 2 * C], bf16)
    nc.sync.dma_start(mlpw[:, :], mlpw_d)
    dts = mlpw[:, 0:S]
    wk1t = mlpw[:, S : S + C]
    wk2td = mlpw[:, S + C : S + C + KK * KK * 2 * C]

    # ---------------- resident x (halo rows zeroed, +-1 elem pad) ----------
    xs = xpool.tile([128, XR * W + 2], bf16)
    nc.vector.memset(xs[:, 0 : W + 1], 0.0)
    nc.vector.memset(xs[:, (XR - 1) * W + 1 : XR * W + 2], 0.0)
    XBLK = 8

    def _xchunks():
        """order the 16 aligned 8-row x chunks by estimated consume time."""
        consume_at = {}
        for stripes, pace in ((STRIPES_A, PACE_A), (STRIPES_B2, PACE_B2),
                              (STRIPES_B3, PACE_B3)):
            t = 0.0
            for r0, n in stripes:
                for r in range(r0, r0 + n):
                    consume_at[r] = t + pace * (r - r0)
                t += pace * n

        def consume(k):
            rows = range(max(0, k - 1), min(H, k + XBLK + 1))
            return min(consume_at[r] for r in rows)

        return sorted(range(0, H, XBLK), key=consume)

    wcbd = const.tile([2 * C, 2 * C], bf16)
    bc2 = const.tile([2 * C, 1], f32)
    _xorder = _xchunks()
    # SP carries the early-consumed chunks; ACT takes K_XACT positions after
    # the MLP section (so the in-order ACT queue does the MLP prelu first).
    _xact = [k for ci, k in enumerate(_xorder) if ci in K_XACT]
    _xsp = [k for k in _xorder if k not in _xact]
    for ci, k in enumerate(_xsp):
        nc.sync.dma_start(
            xs[:, (k + 1) * W + 1 : (k + XBLK + 1) * W + 1],
            x_d[:, k * W : (k + XBLK) * W],
        )
        if ci == 3:
            nc.sync.dma_start(wcbd[:, :], wcbd_d)
            nc.sync.dma_start(bc2[:, :], bc_d)

    # dw output (lrelu'd, bf16) == rhs of the 1x1 conv
    dwout = dwp.tile([128, H * W], bf16)



    # ---------------- kernel-generating MLP ----------------
    hid_ps = po.tile([C, S], f32, tag="po")
    nc.tensor.matmul(hid_ps[:, :], lhsT=wk1t, rhs=dts, start=True, stop=True)
    hid_sb = const.tile([C, S], bf16)
    nc.scalar.activation(hid_sb[:, :], hid_ps[:, :], PRELU, alpha=0.1)

    kcols = const.tile([2 * C, KK * KK], f32)
    kps = po.tile([2 * C, KK * KK], f32, tag="po")
    for t in range(KK * KK):
        nc.tensor.matmul(
            kps[0:C, t : t + 1],
            lhsT=wk2td[:, t * 128 : t * 128 + C],
            rhs=hid_sb[:, 0:1],
            start=True, stop=True,
        )
        nc.tensor.matmul(
            kps[C : 2 * C, t : t + 1],
            lhsT=wk2td[:, t * 128 + C : (t + 1) * 128],
            rhs=hid_sb[:, 1:2],
            start=True, stop=True,
        )
    nc.vector.tensor_copy(kcols[:, :], kps[:, :])

    # per-tap diagonal weights for the PE region
    id_i = const.tile([128, 128], mybir.dt.int32)
    nc.gpsimd.iota(id_i[:, :], pattern=[[1, 128]], base=0, channel_multiplier=-1)
    idf = const.tile([128, 128], bf16)
    nc.vector.tensor_scalar(idf[:, :], id_i[:, :], 0, None, A.is_equal)
    diag = const.tile([128, KK * KK * 128], bf16)
    for t in range(KK * KK):
        nc.vector.tensor_scalar_mul(
            diag[:, t * 128 : (t + 1) * 128], idf[:, :], kcols[:, t : t + 1]
        )
    # negated diagonals for the W-edge wrap corrections (taps dj=0 and dj=2)
    negk = const.tile([2 * C, KK * KK], f32)
    nc.vector.tensor_scalar_mul(negk[:, :], kcols[:, :], -1.0)
    negdiag = const.tile([128, 6 * 128], bf16)
    for j, (di, dj) in enumerate(CORR):
        t = di * KK + dj
        nc.vector.tensor_scalar_mul(
            negdiag[:, j * 128 : (j + 1) * 128], idf[:, :], negk[:, t : t + 1]
        )

    for k in _xact:
        nc.scalar.dma_start(
            xs[:, (k + 1) * W + 1 : (k + XBLK + 1) * W + 1],
            x_d[:, k * W : (k + XBLK) * W],
        )

    xs3 = xs[:, 1 : 1 + XR * W].rearrange("p (r w) -> p r w", w=W)  # [128, XR, W]
    dw3 = dwout[:, :].rearrange("p (r w) -> p r w", w=W)

    # ---------------- region emitters ----------------
    def emit_A():
        """PE: 9 full-width taps + 6 strided W-edge corrections per 4-row
        bank; ACT Prelu evacuates to dwout."""
        banks = [r0 + 4 * i for r0, n in STRIPES_A for i in range(n // 4)]
        emit_A.bi = 0
        emit_A.nb = len(banks)
        for rb in banks:
            if True:
                ps = pdw.tile([128, 512], f32, tag="pdw", name=f"pdw{rb}")
                bank = ps[:, :]
                for i, (di, dj) in enumerate(TAPS[:-1]):
                    t = di * KK + dj
                    base = (rb + di) * W + dj
                    nc.tensor.matmul(
                        bank[:, :],
                        lhsT=diag[:, t * 128 : (t + 1) * 128],
                        rhs=xs[:, base : base + 512],
                        start=(i == 0),
                        stop=False,
                    )
                for j, (di, dj) in enumerate(CORR):
                    if dj == 0:
                        dst = bank[:, 0 : 512 : W]
                        c0 = (rb + di) * W
                    else:
                        dst = bank[:, W - 1 : 512 : W]
                        c0 = (rb + di + 1) * W + 1
                    nc.tensor.matmul(
                        dst,
                        lhsT=negdiag[:, j * 128 : (j + 1) * 128],
                        rhs=xs[:, c0 : c0 + 3 * W + 1 : W],
                        start=False,
                        stop=False,
                    )
                di, dj = TAPS[-1]
                t = di * KK + dj
                base = (rb + di) * W + dj
                nc.tensor.matmul(
                    bank[:, :],
                    lhsT=diag[:, t * 128 : (t + 1) * 128],
                    rhs=xs[:, base : base + 512],
                    start=False,
                    stop=True,
                )
            emit_A.bi += 1
            if K_ASPLIT and emit_A.bi > emit_A.nb - K_ASPLIT:
                nc.scalar.activation(
                    dwout[:, rb * W : rb * W + 256], ps[:, 0:256], PRELU, alpha=0.1
                )
                nc.vector.scalar_tensor_tensor(
                    dwout[:, rb * W + 256 : (rb + 4) * W], ps[:, 256:512], 0.1,
                    ps[:, 256:512], op0=A.mult, op1=A.max,
                )
            else:
                nc.scalar.activation(
                    dwout[:, rb * W : (rb + 4) * W], ps[:, :], PRELU, alpha=0.1
                )
            yield rb, 4

    def emit_B2():
        """DVE: scale-mul into tmp (4x), tensor-tensor add into dwout (2x);
        lrelu on ACT."""
        for ci, (r0, n) in enumerate(STRIPES_B2):
            last = ci == len(STRIPES_B2) - 1
            dst = dwout[:, r0 * W : (r0 + n) * W]
            t0 = 0 * KK + 1
            nc.vector.tensor_scalar_mul(
                dst, xs[:, r0 * W + 1 : (r0 + n) * W + 1], kcols[:, t0 : t0 + 1]
            )
            for di, dj in TAPS:
                if (di, dj) == (0, 1):
                    continue
                t = di * KK + dj
                w_lo, s_lo, ncol = _tap_geom(dj)
                tm = tmpp.tile([128, 16 * W], bf16, tag="b2t", name=f"tm{r0}_{t}")
                tm3 = tm[:, :].rearrange("p (r w) -> p r w", w=W)
                src = xs3[:, r0 + di : r0 + di + n, s_lo : s_lo + ncol]
                nc.vector.tensor_scalar_mul(tm3[:, 0:n, 0:ncol], src, kcols[:, t : t + 1])
                nc.vector.tensor_tensor(
                    dw3[:, r0 : r0 + n, w_lo : w_lo + ncol],
                    dw3[:, r0 : r0 + n, w_lo : w_lo + ncol],
                    tm3[:, 0:n, 0:ncol],
                    A.add,
                )
            _lrelu_maybe_split(nc, dst, n,
                               K_LSPLIT and ci >= len(STRIPES_B2) - K_LSPLIT)
            yield r0, n

    def emit_B3():
        """Pool: scale-mul into tmp (TensorScalarPtr) + tensor-tensor add
        into dwout; lrelu on ACT."""
        B3MAX = max(n for _, n in STRIPES_B3)
        for ci, (r0, n) in enumerate(STRIPES_B3):
            dst = dwout[:, r0 * W : (r0 + n) * W]
            t0 = 0 * KK + 1
            nc.gpsimd.tensor_scalar_mul(
                dst, xs[:, r0 * W + 1 : (r0 + n) * W + 1], kcols[:, t0 : t0 + 1]
            )
            for di, dj in TAPS:
                if (di, dj) == (0, 1):
                    continue
                t = di * KK + dj
                w_lo, s_lo, ncol = _tap_geom(dj)
                pt = sbp.tile([128, B3MAX * W], bf16, tag="p3t", name=f"pt{r0}_{t}")
                pt3 = pt[:, :].rearrange("p (r w) -> p r w", w=W)
                src = xs3[:, r0 + di : r0 + di + n, s_lo : s_lo + ncol]
                nc.gpsimd.tensor_scalar_mul(
                    pt3[:, 0:n, 0:ncol], src, kcols[:, t : t + 1]
                )
                nc.gpsimd.tensor_tensor(
                    dw3[:, r0 : r0 + n, w_lo : w_lo + ncol],
                    dw3[:, r0 : r0 + n, w_lo : w_lo + ncol],
                    pt3[:, 0:n, 0:ncol],
                    A.add,
                )
            _lrelu_maybe_split(nc, dst, n,
                               K_LSPLIT and ci >= len(STRIPES_B3) - K_LSPLIT)
            yield r0, n

    # ------- merged emission: dw chunks + 1x1 groups by virtual clocks -----
    def emit_group(g):
        rb = g * 8
        emit_group.count += 1
        if K_TAILFINE and emit_group.count > H // 8 - K_TAILFINE:
            # fine-grained drain: two independent 4-row units, evacs
            # alternating ACT/DVE, separate output DMAs
            for half in range(2):
                off = (rb + 4 * half) * W
                psf = po.tile([128, 512], f32, tag="po", name=f"pof{g}_{half}")
                nc.tensor.matmul(
                    psf[:, :], lhsT=wcbd[:, :], rhs=dwout[:, off : off + 512],
                    start=True, stop=True,
                )
                obf = obp.tile([128, 512], bf16, tag="ob", name=f"obf{g}_{half}")
                if half == 0:
                    nc.scalar.activation(obf[:, :], psf[:, :], IDENT, bias=bc2[:, 0:1])
                else:
                    nc.vector.tensor_scalar_add(obf[:, :], psf[:, :], bc2[:, 0:1])
                nc.sync.dma_start(out_d[:, off : off + 512], obf[:, :])
            return
        ps1 = po.tile([128, 1024], f32, tag="po", name=f"po{g}")
        for half in range(2):
            off = (rb + 4 * half) * W
            nc.tensor.matmul(
                ps1[:, half * 512 : (half + 1) * 512],
                lhsT=wcbd[:, :],
                rhs=dwout[:, off : off + 512],
                start=True, stop=True,
            )
        ob = obp.tile([128, 1024], bf16, tag="ob", name=f"ob{g}")
        if K_TAILSPLIT and emit_group.count > H // 8 - K_TAILSPLIT:
            # split the evac: ACT and DVE each do one 512 half concurrently
            nc.scalar.activation(ob[:, 0:512], ps1[:, 0:512], IDENT, bias=bc2[:, 0:1])
            nc.vector.tensor_scalar_add(ob[:, 512:1024], ps1[:, 512:1024], bc2[:, 0:1])
        else:
            nc.scalar.activation(ob[:, :], ps1[:, :], IDENT, bias=bc2[:, 0:1])
        if K_ODMA and emit_group.count > H // 8 - K_ODMA and emit_group.count % 2 == K_ODMAPAR:
            nc.scalar.dma_start(out_d[:, rb * W : (rb + 8) * W], ob[:, :])
        else:
            nc.sync.dma_start(out_d[:, rb * W : (rb + 8) * W], ob[:, :])

    emit_group.count = 0
    FILL = K_FILL  # rough ns before first dw work can start
    import random as _random
    _rng = _random.Random(K_SEED) if K_SEED else None
    regions = [
        (emit_A(), PACE_A),
        (emit_B2(), PACE_B2),
        (emit_B3(), PACE_B3),
    ]
    clocks = [FILL] * len(regions)
    done_at: dict[int, float] = {}  # 4-row unit index -> est completion
    pending: list[tuple[float, int]] = []  # (ready est, 1x1 group)
    queued: set[int] = set()
    idx = list(range(len(regions)))
    while idx or pending:
        tnext = min((clocks[j] for j in idx), default=1e18)
        if pending and pending[0][0] <= tnext:
            _, g = pending.pop(0)
            emit_group(g)
            continue
        if not idx:
            continue
        i = min(idx, key=lambda j: clocks[j])
        ggen, pace = regions[i]
        try:
            r0, n = next(ggen)
        except StopIteration:
            idx.remove(i)
            continue
        assert r0 % 4 == 0 and n % 4 == 0
        jit = 1.0 + (_rng.uniform(-K_AMP, K_AMP) if _rng else 0.0)
        clocks[i] += pace * n * jit
        for u in range(r0 // 4, (r0 + n) // 4):
            done_at[u] = clocks[i]
        for g in range(H // 8):
            if g in queued:
                continue
            if 2 * g in done_at and 2 * g + 1 in done_at:
                queued.add(g)
                pending.append((max(done_at[2 * g], done_at[2 * g + 1]), g))
        pending.sort()
    assert len(queued) == H // 8, f"unemitted 1x1 groups: {sorted(set(range(16)) - queued)}"


# ---------------------------------------------------------------------------
# host-side entry point
# ---------------------------------------------------------------------------

_PROGRAM_CACHE: dict[str, bass.Bass] = {}


def _get_program() -> bass.Bass:
    if "p" not in _PROGRAM_CACHE:
        _PROGRAM_CACHE["p"] = build_program()
    return _PROGRAM_CACHE["p"]


def _host_prep(inputs: dict):
    import ml_dtypes

    x = np.asarray(inputs["x"], dtype=np.float32)
    d = np.asarray(inputs["d"], dtype=np.float32)
    Wk1 = np.asarray(inputs["Wk1"], dtype=np.float32)
    Wk2 = np.asarray(inputs["Wk2"], dtype=np.float32)
    Wc = np.asarray(inputs["Wc"], dtype=np.float32)
    bc = np.asarray(inputs["bc"], dtype=np.float32)

    wk1t = np.ascontiguousarray(Wk1.T)
    w = Wk2.reshape(C, KK * KK, C).transpose(2, 1, 0)  # (j, t, c)
    wk2td = np.ascontiguousarray(
        np.concatenate([w, w], axis=2).reshape(C, KK * KK * 2 * C)
    )
    wct = np.ascontiguousarray(Wc.T)
    wcbd = np.zeros((2 * C, 2 * C), np.float32)
    wcbd[0:C, 0:C] = wct
    wcbd[C:, C:] = wct
    wcbd = wcbd.astype(ml_dtypes.bfloat16)
    bc2 = np.ascontiguousarray(np.concatenate([bc, bc]).reshape(2 * C, 1))

    xcast = x.astype(ml_dtypes.bfloat16)

    in_maps = []
    for i in range(NCORES):
        xsamp = np.ascontiguousarray(xcast[S * i : S * (i + 1)].reshape(S * C, H * W))
        dT = np.ascontiguousarray(d[S * i : S * (i + 1)].T)
        mlpw = np.concatenate([dT, wk1t, wk2td], axis=1).astype(ml_dtypes.bfloat16)
        in_maps.append(
            {
                "x": xsamp,
                "mlpw": np.ascontiguousarray(mlpw),
                "wcbd": wcbd,
                "bc2": bc2,
            }
        )
    return in_maps


def run_on_hw(inputs: dict, **kwargs):
    from concourse.bass_utils import run_bass_kernel_spmd

    nc = _get_program()
    in_maps = _host_prep(inputs)
    res = run_bass_kernel_spmd(nc, in_maps, core_ids=list(range(NCORES)), **kwargs)
    outs = res.results
    B = S * NCORES
    out = np.empty((B, C, H, W), dtype=np.float32)
    for i in range(NCORES):
        out[S * i : S * (i + 1)] = (
            outs[i]["out"].astype(np.float32).reshape(S, C, H, W)
        )
    return out, res


def kernel(**inputs) -> np.ndarray:
    out, _ = run_on_hw(inputs)
    return out


if __name__ == "__main__":
    nc = build_program()
    print("program built OK")


# revision 11
# speedup vs baseline: 2.9038x; 1.0012x over previous
"""Trainium2 Bass kernel for nn_DA_conv: per-sample dynamic depthwise 3x3 conv
(+LeakyReLU) followed by a 1x1 pointwise conv, with the 3x3 kernels produced by
a small per-sample MLP.

Strategy (8 NeuronCores, batch data-parallel, 2 samples per core):
  - SBUF partition p = (sample s = p//64, channel c = p%64); the feature map
    is resident in bf16 with top/bottom halo rows plus a one-element pad on
    each end (no W padding: edge columns are handled by full-width taps plus
    strided correction matmuls / partial-width vector ops).
  - The image rows are split into three regions, one per compute engine, so
    the depthwise conv runs on PE, DVE and GPSIMD concurrently:
      A  : PE diagonal matmuls (9 full-width PSUM-accumulated taps + 6
           strided W-edge corrections per 4-row bank), ACT Prelu evac.
      B2 : DVE per-tap scale-muls (4x mode) + tensor-tensor adds.
      B3 : gpsimd scalar_tensor_tensor MAC chain.
    LeakyReLU for B2/B3 runs on ACT (Prelu), interleaved by estimated
    readiness so the in-order ACT queue never stalls the PSUM pipeline.
  - 1x1 conv: one 128x128 block-diagonal matmul per PSUM bank covers both
    samples at once; ACT evacuates with the bias via Identity+bias.
  - DMA: input + output both on the otherwise-idle SP queue; compute engines
    never issue DMAs.
"""

import sys

sys.path.insert(0, "/opt/trn_rl_repo")

from contextlib import ExitStack

import numpy as np

import concourse.bacc as bacc
import concourse.bass as bass
import concourse.mybir as mybir
import concourse.tile as tile

S = 2            # samples per core
C = 64           # channels
H = W = 128      # spatial
KK = 3           # conv kernel size
NCORES = 8
XR = H + 2       # resident rows incl halo

f32 = mybir.dt.float32
bf16 = mybir.dt.bfloat16
A = mybir.AluOpType
PRELU = mybir.ActivationFunctionType.Prelu
IDENT = mybir.ActivationFunctionType.Identity

# Row stripes per engine (cover 0..128, multiples of 4).
# Interleaved so 1x1-group completions stagger instead of bunching at the end.
STRIPES_A = [(0, 16), (28, 16), (56, 16), (84, 16), (124, 4)]   # PE, 68 rows
STRIPES_B2 = [(44, 12), (16, 12), (112, 12), (72, 4)]           # DVE, 40 rows
STRIPES_B3 = [(100, 12), (76, 8)]                               # Pool, 20 rows
R_A = sum(n for _, n in STRIPES_A)
R_B2 = sum(n for _, n in STRIPES_B2)
R_B3 = sum(n for _, n in STRIPES_B3)

# virtual-clock pace estimates (ns per image row) for emission ordering
PACE_A = 450.0
PACE_B2 = 890.0
PACE_B3 = 1850.0
K_FILL = 3000.0
K_XACT = [6, 11]     # consume-order chunk indices carried by the ACT queue
K_TAILSPLIT = 4      # last-N 1x1 groups: evac halves split across ACT/DVE
K_ASPLIT = 0
K_LSPLIT = 1    # last B2/B3 chunk: lrelu halves split across ACT/DVE
K_TAILFINE = 0
K_ODMA = 2      # last group: out DMA via the idle ACT queue
K_ODMAPAR = 0
K_SEED = 5    # emission-order jitter seed (best of 50+ seed/amplitude search)
K_AMP = 0.40

def _lrelu_maybe_split(nc, dst, n, split):
    if split:
        half = (n * W) // 2
        nc.scalar.activation(dst[:, 0:half], dst[:, 0:half],
                             mybir.ActivationFunctionType.Prelu, alpha=0.1)
        nc.vector.scalar_tensor_tensor(
            dst[:, half:], dst[:, half:], 0.1, dst[:, half:],
            op0=A.mult, op1=A.max)
    else:
        nc.scalar.activation(dst, dst, mybir.ActivationFunctionType.Prelu, alpha=0.1)

TAPS = [(di, dj) for di in range(KK) for dj in range(KK)]
CORR = [(di, dj) for dj in (0, 2) for di in range(KK)]


def _tap_geom(dj):
    """(out_col_lo, src_col_lo, ncols) for a horizontal tap shift."""
    if dj == 0:
        return 1, 0, W - 1
    if dj == 1:
        return 0, 0, W
    return 0, 1, W - 1


def build_program() -> bass.Bass:
    nc = bacc.Bacc("TRN2", target_bir_lowering=False, debug=False)

    x_d = nc.dram_tensor("x", [S * C, H * W], bf16, kind="ExternalInput").ap()
    # mlpw = [dT (2) | Wk1.T (64) | wk2td (1152)] in one tensor -> one DMA
    # (wk2td[j, t*128 + s*64 + c] = Wk2[c*9 + t, j], tap-major, sample-dup)
    mlpw_d = nc.dram_tensor(
        "mlpw", [C, S + C + KK * KK * 2 * C], bf16, kind="ExternalInput"
    ).ap()
    wcbd_d = nc.dram_tensor("wcbd", [2 * C, 2 * C], bf16, kind="ExternalInput").ap()
    bc_d = nc.dram_tensor("bc2", [2 * C, 1], f32, kind="ExternalInput").ap()
    out_d = nc.dram_tensor("out", [S * C, H * W], bf16, kind="ExternalOutput").ap()

    with tile.TileContext(nc) as tc, ExitStack() as ctx:
        _body(ctx, tc, x_d, mlpw_d, wcbd_d, bc_d, out_d)
    nc.compile()
    return nc


def _body(ctx, tc, x_d, mlpw_d, wcbd_d, bc_d, out_d):
    nc = tc.nc
    const = ctx.enter_context(tc.tile_pool(name="const", bufs=1))
    xpool = ctx.enter_context(tc.tile_pool(name="xs", bufs=1))
    dwp = ctx.enter_context(tc.tile_pool(name="dw", bufs=1))
    tmpp = ctx.enter_context(tc.tile_pool(name="tmp", bufs=2))
    sbp = ctx.enter_context(tc.tile_pool(name="sb", bufs=2))
    obp = ctx.enter_context(tc.tile_pool(name="ob", bufs=3))
    pdw = ctx.enter_context(tc.tile_pool(name="pdw", bufs=2, space="PSUM"))
    po = ctx.enter_context(tc.tile_pool(name="po", bufs=3, space="PSUM"))

    # ---------------- const loads (qSP, before x) ----------------
    mlpw = const.tile([C, S + C + KK * KK * 2 * C], bf16)
    nc.sync.dma_start(mlpw[:, :], mlpw_d)
    dts = mlpw[:, 0:S]
    wk1t = mlpw[:, S : S + C]
    wk2td = mlpw[:, S + C : S + C + KK * KK * 2 * C]

    # ---------------- resident x (halo rows zeroed, +-1 elem pad) ----------
    xs = xpool.tile([128, XR * W + 2], bf16)
    nc.vector.memset(xs[:, 0 : W + 1], 0.0)
    nc.vector.memset(xs[:, (XR - 1) * W + 1 : XR * W + 2], 0.0)
    XBLK = 8

    def _xchunks():
        """order the 16 aligned 8-row x chunks by estimated consume time."""
        consume_at = {}
        for stripes, pace in ((STRIPES_A, PACE_A), (STRIPES_B2, PACE_B2),
                              (STRIPES_B3, PACE_B3)):
            t = 0.0
            for r0, n in stripes:
                for r in range(r0, r0 + n):
                    consume_at[r] = t + pace * (r - r0)
                t += pace * n

        def consume(k):
            rows = range(max(0, k - 1), min(H, k + XBLK + 1))
            return min(consume_at[r] for r in rows)

        return sorted(range(0, H, XBLK), key=consume)

    wcbd = const.tile([2 * C, 2 * C], bf16)
    bc2 = const.tile([2 * C, 1], f32)
    _xorder = _xchunks()
    # SP carries the early-consumed chunks; ACT takes K_XACT positions after
    # the MLP section (so the in-order ACT queue does the MLP prelu first).
    _xact = [k for ci, k in enumerate(_xorder) if ci in K_XACT]
    _xsp = [k for k in _xorder if k not in _xact]
    for ci, k in enumerate(_xsp):
        nc.sync.dma_start(
            xs[:, (k + 1) * W + 1 : (k + XBLK + 1) * W + 1],
            x_d[:, k * W : (k + XBLK) * W],
        )
        if ci == 3:
            nc.sync.dma_start(wcbd[:, :], wcbd_d)
            nc.sync.dma_start(bc2[:, :], bc_d)

    # dw output (lrelu'd, bf16) == rhs of the 1x1 conv
    dwout = dwp.tile([128, H * W], bf16)



    # ---------------- kernel-generating MLP ----------------
    hid_ps = po.tile([C, S], f32, tag="po")
    nc.tensor.matmul(hid_ps[:, :], lhsT=wk1t, rhs=dts, start=True, stop=True)
    hid_sb = const.tile([C, S], bf16)
    nc.scalar.activation(hid_sb[:, :], hid_ps[:, :], PRELU, alpha=0.1)

    kcols = const.tile([2 * C, KK * KK], f32)
    kps = po.tile([2 * C, KK * KK], f32, tag="po")
    for t in range(KK * KK):
        nc.tensor.matmul(
            kps[0:C, t : t + 1],
            lhsT=wk2td[:, t * 128 : t * 128 + C],
            rhs=hid_sb[:, 0:1],
            start=True, stop=True,
        )
        nc.tensor.matmul(
            kps[C : 2 * C, t : t + 1],
            lhsT=wk2td[:, t * 128 + C : (t + 1) * 128],
            rhs=hid_sb[:, 1:2],
            start=True, stop=True,
        )
    nc.vector.tensor_copy(kcols[:, :], kps[:, :])

    # per-tap diagonal weights for the PE region
    id_i = const.tile([128, 128], mybir.dt.int32)
    nc.gpsimd.iota(id_i[:, :], pattern=[[1, 128]], base=0, channel_multiplier=-1)
    idf = const.tile([128, 128], bf16)
    nc.vector.tensor_scalar(idf[:, :], id_i[:, :], 0, None, A.is_equal)
    diag = const.tile([128, KK * KK * 128], bf16)
    for t in range(KK * KK):
        nc.vector.tensor_scalar_mul(
            diag[:, t * 128 : (t + 1) * 128], idf[:, :], kcols[:, t : t + 1]
        )
    # negated diagonals for the W-edge wrap corrections (taps dj=0 and dj=2)
    negk = const.tile([2 * C, KK * KK], f32)
    nc.vector.tensor_scalar_mul(negk[:, :], kcols[:, :], -1.0)
    negdiag = const.tile([128, 6 * 128], bf16)
    for j, (di, dj) in enumerate(CORR):
        t = di * KK + dj
        nc.vector.tensor_scalar_mul(
            negdiag[:, j * 128 : (j + 1) * 128], idf[:, :], negk[:, t : t + 1]
        )

    for k in _xact:
        nc.scalar.dma_start(
            xs[:, (k + 1) * W + 1 : (k + XBLK + 1) * W + 1],
            x_d[:, k * W : (k + XBLK) * W],
        )

    xs3 = xs[:, 1 : 1 + XR * W].rearrange("p (r w) -> p r w", w=W)  # [128, XR, W]
    dw3 = dwout[:, :].rearrange("p (r w) -> p r w", w=W)

    # ---------------- region emitters ----------------
    def emit_A():
        """PE: 9 full-width taps + 6 strided W-edge corrections per 4-row
        bank; ACT Prelu evacuates to dwout."""
        banks = [r0 + 4 * i for r0, n in STRIPES_A for i in range(n // 4)]
        emit_A.bi = 0
        emit_A.nb = len(banks)
        for rb in banks:
            if True:
                ps = pdw.tile([128, 512], f32, tag="pdw", name=f"pdw{rb}")
                bank = ps[:, :]
                for i, (di, dj) in enumerate(TAPS[:-1]):
                    t = di * KK + dj
                    base = (rb + di) * W + dj
                    nc.tensor.matmul(
                        bank[:, :],
                        lhsT=diag[:, t * 128 : (t + 1) * 128],
                        rhs=xs[:, base : base + 512],
                        start=(i == 0),
                        stop=False,
                    )
                for j, (di, dj) in enumerate(CORR):
                    if dj == 0:
                        dst = bank[:, 0 : 512 : W]
                        c0 = (rb + di) * W
                    else:
                        dst = bank[:, W - 1 : 512 : W]
                        c0 = (rb + di + 1) * W + 1
                    nc.tensor.matmul(
                        dst,
                        lhsT=negdiag[:, j * 128 : (j + 1) * 128],
                        rhs=xs[:, c0 : c0 + 3 * W + 1 : W],
                        start=False,
                        stop=False,
                    )
                di, dj = TAPS[-1]
                t = di * KK + dj
                base = (rb + di) * W + dj
                nc.tensor.matmul(
                    bank[:, :],
                    lhsT=diag[:, t * 128 : (t + 1) * 128],
                    rhs=xs[:, base : base + 512],
                    start=False,
                    stop=True,
                )
            emit_A.bi += 1
            if K_ASPLIT and emit_A.bi > emit_A.nb - K_ASPLIT:
                nc.scalar.activation(
                    dwout[:, rb * W : rb * W + 256], ps[:, 0:256], PRELU, alpha=0.1
                )
                nc.vector.scalar_tensor_tensor(
                    dwout[:, rb * W + 256 : (rb + 4) * W], ps[:, 256:512], 0.1,
                    ps[:, 256:512], op0=A.mult, op1=A.max,
                )
            else:
                nc.scalar.activation(
                    dwout[:, rb * W : (rb + 4) * W], ps[:, :], PRELU, alpha=0.1
                )
            yield rb, 4

    def emit_B2():
        """DVE: scale-mul into tmp (4x), tensor-tensor add into dwout (2x);
        lrelu on ACT."""
        for ci, (r0, n) in enumerate(STRIPES_B2):
            last = ci == len(STRIPES_B2) - 1
            dst = dwout[:, r0 * W : (r0 + n) * W]
            t0 = 0 * KK + 1
            nc.vector.tensor_scalar_mul(
                dst, xs[:, r0 * W + 1 : (r0 + n) * W + 1], kcols[:, t0 : t0 + 1]
            )
            for di, dj in TAPS:
                if (di, dj) == (0, 1):
                    continue
                t = di * KK + dj
                w_lo, s_lo, ncol = _tap_geom(dj)
                tm = tmpp.tile([128, 16 * W], bf16, tag="b2t", name=f"tm{r0}_{t}")
                tm3 = tm[:, :].rearrange("p (r w) -> p r w", w=W)
                src = xs3[:, r0 + di : r0 + di + n, s_lo : s_lo + ncol]
                nc.vector.tensor_scalar_mul(tm3[:, 0:n, 0:ncol], src, kcols[:, t : t + 1])
                nc.vector.tensor_tensor(
                    dw3[:, r0 : r0 + n, w_lo : w_lo + ncol],
                    dw3[:, r0 : r0 + n, w_lo : w_lo + ncol],
                    tm3[:, 0:n, 0:ncol],
                    A.add,
                )
            _lrelu_maybe_split(nc, dst, n,
                               K_LSPLIT and ci >= len(STRIPES_B2) - K_LSPLIT)
            yield r0, n

    def emit_B3():
        """Pool: scale-mul into tmp (TensorScalarPtr) + tensor-tensor add
        into dwout; lrelu on ACT."""
        B3MAX = max(n for _, n in STRIPES_B3)
        for ci, (r0, n) in enumerate(STRIPES_B3):
            dst = dwout[:, r0 * W : (r0 + n) * W]
            t0 = 0 * KK + 1
            nc.gpsimd.tensor_scalar_mul(
                dst, xs[:, r0 * W + 1 : (r0 + n) * W + 1], kcols[:, t0 : t0 + 1]
            )
            for di, dj in TAPS:
                if (di, dj) == (0, 1):
                    continue
                t = di * KK + dj
                w_lo, s_lo, ncol = _tap_geom(dj)
                pt = sbp.tile([128, B3MAX * W], bf16, tag="p3t", name=f"pt{r0}_{t}")
                pt3 = pt[:, :].rearrange("p (r w) -> p r w", w=W)
                src = xs3[:, r0 + di : r0 + di + n, s_lo : s_lo + ncol]
                nc.gpsimd.tensor_scalar_mul(
                    pt3[:, 0:n, 0:ncol], src, kcols[:, t : t + 1]
                )
                nc.gpsimd.tensor_tensor(
                    dw3[:, r0 : r0 + n, w_lo : w_lo + ncol],
                    dw3[:, r0 : r0 + n, w_lo : w_lo + ncol],
                    pt3[:, 0:n, 0:ncol],
                    A.add,
                )
            _lrelu_maybe_split(nc, dst, n,
                               K_LSPLIT and ci >= len(STRIPES_B3) - K_LSPLIT)
            yield r0, n

    # ------- merged emission: dw chunks + 1x1 groups by virtual clocks -----
    def emit_group(g):
        rb = g * 8
        emit_group.count += 1
        if K_TAILFINE and emit_group.count > H // 8 - K_TAILFINE:
            # fine-grained drain: two independent 4-row units, evacs
            # alternating ACT/DVE, separate output DMAs
            for half in range(2):
                off = (rb + 4 * half) * W
                psf = po.tile([128, 512], f32, tag="po", name=f"pof{g}_{half}")
                nc.tensor.matmul(
                    psf[:, :], lhsT=wcbd[:, :], rhs=dwout[:, off : off + 512],
                    start=True, stop=True,
                )
                obf = obp.tile([128, 512], bf16, tag="ob", name=f"obf{g}_{half}")
                if half == 0:
                    nc.scalar.activation(obf[:, :], psf[:, :], IDENT, bias=bc2[:, 0:1])
                else:
                    nc.vector.tensor_scalar_add(obf[:, :], psf[:, :], bc2[:, 0:1])
                nc.sync.dma_start(out_d[:, off : off + 512], obf[:, :])
            return
        ps1 = po.tile([128, 1024], f32, tag="po", name=f"po{g}")
        for half in range(2):
            off = (rb + 4 * half) * W
            nc.tensor.matmul(
                ps1[:, half * 512 : (half + 1) * 512],
                lhsT=wcbd[:, :],
                rhs=dwout[:, off : off + 512],
                start=True, stop=True,
            )
        ob = obp.tile([128, 1024], bf16, tag="ob", name=f"ob{g}")
        if K_TAILSPLIT and emit_group.count > H // 8 - K_TAILSPLIT:
            # split the evac: ACT and DVE each do one 512 half concurrently
            nc.scalar.activation(ob[:, 0:512], ps1[:, 0:512], IDENT, bias=bc2[:, 0:1])
            nc.vector.tensor_scalar_add(ob[:, 512:1024], ps1[:, 512:1024], bc2[:, 0:1])
        else:
            nc.scalar.activation(ob[:, :], ps1[:, :], IDENT, bias=bc2[:, 0:1])
        if K_ODMA and emit_group.count > H // 8 - K_ODMA and emit_group.count % 2 == K_ODMAPAR:
            nc.scalar.dma_start(out_d[:, rb * W : (rb + 8) * W], ob[:, :])
        else:
            nc.sync.dma_start(out_d[:, rb * W : (rb + 8) * W], ob[:, :])

    emit_group.count = 0
    FILL = K_FILL  # rough ns before first dw work can start
    import random as _random
    _rng = _random.Random(K_SEED) if K_SEED else None
    regions = [
        (emit_A(), PACE_A),
        (emit_B2(), PACE_B2),
        (emit_B3(), PACE_B3),
    ]
    clocks = [FILL] * len(regions)
    done_at: dict[int, float] = {}  # 4-row unit index -> est completion
    pending: list[tuple[float, int]] = []  # (ready est, 1x1 group)
    queued: set[int] = set()
    idx = list(range(len(regions)))
    while idx or pending:
        tnext = min((clocks[j] for j in idx), default=1e18)
        if pending and pending[0][0] <= tnext:
            _, g = pending.pop(0)
            emit_group(g)
            continue
        if not idx:
            continue
        i = min(idx, key=lambda j: clocks[j])
        ggen, pace = regions[i]
        try:
            r0, n = next(ggen)
        except StopIteration:
            idx.remove(i)
            continue
        assert r0 % 4 == 0 and n % 4 == 0
        jit = 1.0 + (_rng.uniform(-K_AMP, K_AMP) if _rng else 0.0)
        clocks[i] += pace * n * jit
        for u in range(r0 // 4, (r0 + n) // 4):
            done_at[u] = clocks[i]
        for g in range(H // 8):
            if g in queued:
                continue
            if 2 * g in done_at and 2 * g + 1 in done_at:
                queued.add(g)
                pending.append((max(done_at[2 * g], done_at[2 * g + 1]), g))
        pending.sort()
    assert len(queued) == H // 8, f"unemitted 1x1 groups: {sorted(set(range(16)) - queued)}"


# ---------------------------------------------------------------------------
# host-side entry point
# ---------------------------------------------------------------------------

_PROGRAM_CACHE: dict[str, bass.Bass] = {}


def _get_program() -> bass.Bass:
    if "p" not in _PROGRAM_CACHE:
        _PROGRAM_CACHE["p"] = build_program()
    return _PROGRAM_CACHE["p"]


def _host_prep(inputs: dict):
    import ml_dtypes

    x = np.asarray(inputs["x"], dtype=np.float32)
    d = np.asarray(inputs["d"], dtype=np.float32)
    Wk1 = np.asarray(inputs["Wk1"], dtype=np.float32)
    Wk2 = np.asarray(inputs["Wk2"], dtype=np.float32)
    Wc = np.asarray(inputs["Wc"], dtype=np.float32)
    bc = np.asarray(inputs["bc"], dtype=np.float32)

    wk1t = np.ascontiguousarray(Wk1.T)
    w = Wk2.reshape(C, KK * KK, C).transpose(2, 1, 0)  # (j, t, c)
    wk2td = np.ascontiguousarray(
        np.concatenate([w, w], axis=2).reshape(C, KK * KK * 2 * C)
    )
    wct = np.ascontiguousarray(Wc.T)
    wcbd = np.zeros((2 * C, 2 * C), np.float32)
    wcbd[0:C, 0:C] = wct
    wcbd[C:, C:] = wct
    wcbd = wcbd.astype(ml_dtypes.bfloat16)
    bc2 = np.ascontiguousarray(np.concatenate([bc, bc]).reshape(2 * C, 1))

    xcast = x.astype(ml_dtypes.bfloat16)

    in_maps = []
    for i in range(NCORES):
        xsamp = np.ascontiguousarray(xcast[S * i : S * (i + 1)].reshape(S * C, H * W))
        dT = np.ascontiguousarray(d[S * i : S * (i + 1)].T)
        mlpw = np.concatenate([dT, wk1t, wk2td], axis=1).astype(ml_dtypes.bfloat16)
        in_maps.append(
            {
                "x": xsamp,
                "mlpw": np.ascontiguousarray(mlpw),
                "wcbd": wcbd,
                "bc2": bc2,
            }
        )
    return in_maps


def run_on_hw(inputs: dict, **kwargs):
    from concourse.bass_utils import run_bass_kernel_spmd

    nc = _get_program()
    in_maps = _host_prep(inputs)
    res = run_bass_kernel_spmd(nc, in_maps, core_ids=list(range(NCORES)), **kwargs)
    outs = res.results
    B = S * NCORES
    out = np.empty((B, C, H, W), dtype=np.float32)
    for i in range(NCORES):
        out[S * i : S * (i + 1)] = (
            outs[i]["out"].astype(np.float32).reshape(S, C, H, W)
        )
    return out, res


def kernel(**inputs) -> np.ndarray:
    out, _ = run_on_hw(inputs)
    return out


if __name__ == "__main__":
    nc = build_program()
    print("program built OK")
